# revision 1
# baseline (speedup 1.0000x reference)
"""Trainium2 Bass kernel for CombinedAdvancedLoss (focal + contrastive +
circularity + consensus), data-parallel over 8 NeuronCores.

Sharding: batch dim B=32 -> 4 items per core for logits/target/masks/
method_preds. features (1024x512) are passed to each core ROLLED by
-core*128 rows, so every core computes the same SPMD program on "its" 128
rows of the 1024x1024 similarity matrix (row sums / logsumexp are invariant
to the column permutation the roll induces; the diagonal lands in local
column block 0 and the positive pair in block 4).

Each core emits a [1,32] vector of linear partial sums; the host combines
them (the only nonlinear cross-core math - IoU ratios and the circularity
formula - acts on a handful of scalars).
"""

import sys

for _p in ("/opt/trn_rl_repo",):
    if _p not in sys.path:
        sys.path.insert(0, _p)

import numpy as np
import ml_dtypes

import concourse.bass as bass
import concourse.tile as tile
from concourse import mybir
from concourse.bass_utils import run_bass_kernel_spmd

import bass_rust as _bass_rust

# ---------------------------------------------------------------------------
# The walrus build in this container rejects >2 sync waits per instruction.
# Post-pass: hoist excess waits onto inserted same-engine NoOps.
_WAIT_CAP = 1


def _split_sync_waits(nc):
    n = 0
    for fn in nc.m.functions:
        for blk in fn.blocks:
            insts = blk.instructions
            i = 0
            while i < len(insts):
                inst = insts[i]
                si = inst.sync_info
                if si is not None and len(si.on_wait) > _WAIT_CAP:
                    waits = list(si.on_wait)
                    keep = waits[-_WAIT_CAP:]
                    extra = waits[:-_WAIT_CAP]
                    nops = []
                    for j in range(0, len(extra), _WAIT_CAP):
                        nop = mybir.InstDrain(
                            name=f"I-wsplit-{n}", engine=inst.engine)
                        n += 1
                        nop.sync_info = _bass_rust.SyncInfo(
                            on_wait=extra[j:j + _WAIT_CAP], on_update=[])
                        nops.append(nop)
                    inst.sync_info = _bass_rust.SyncInfo(
                        on_wait=keep, on_update=list(si.on_update))
                    for k, nop in enumerate(nops):
                        insts.insert(i + k, nop)
                    i += len(nops)
                i += 1
# ---------------------------------------------------------------------------

F32 = mybir.dt.float32
BF16 = mybir.dt.bfloat16
I32 = mybir.dt.int32
AF = mybir.ActivationFunctionType
OP = mybir.AluOpType
AX = mybir.AxisListType

NCORES = 8
B, C, H, W = 32, 8, 256, 256
BP = B // NCORES          # batch items per core (4)
HW = H * W                # 65536
FD = BP * HW // 128       # free dim of a full-core tile (2048)
XB = HW // 128            # free dim of one plane slice (512)
BF, DF = 1024, 512        # features shape
TEMP = 0.07
GAMMA_SCALE = 0.25        # ALPHA (0.25 for every class) * W_FOCAL
NPART = 32                # width of the per-core partials vector

# partials vector layout
K_FOCAL = 0               # sum 0.25*(1-p)^2 * ce
K_CONTRAST = 1            # sum (lse - pos) over this core's 128 rows
K_AREA = 2                # 4 cols: per-b mask area
K_EX = 6                  # 4 cols: per-b sum |dm/dh|
K_EY = 10                 # 8 cols: per-b (x2 chunks) sum |dm/dw|
K_S = 18                  # 3 cols: per-method sum of preds
K_I = 21                  # 3 cols: per-pair sum pi*pj  (01, 02, 12)
K_EXB = 26                # 4 cols: per-b boundary |m[128]-m[127]|


def _build_nc():
    nc = bass.Bass()

    lg = nc.declare_dram_parameter("lg", [BP, C, 128, XB], F32, isOutput=False)
    tg = nc.declare_dram_parameter("tg", [BP, 128, XB], I32, isOutput=False)
    mk = nc.declare_dram_parameter("mk", [BP, 2, 128, 256], F32, isOutput=False)
    mp = nc.declare_dram_parameter("mp", [3, BP, 128, XB], F32, isOutput=False)
    ft = nc.declare_dram_parameter("ft", [8, 128, DF], F32, isOutput=False)
    idf = nc.declare_dram_parameter("idf", [128, 128], F32, isOutput=False)
    idb = nc.declare_dram_parameter("idb", [128, 128], BF16, isOutput=False)
    zm = nc.declare_dram_parameter("zm", [128, 128], F32, isOutput=False)
    mb = nc.declare_dram_parameter("mb", [1, BP, 2, 256], F32, isOutput=False)
    out = nc.declare_dram_parameter("partials", [1, NPART], F32, isOutput=True)

    with tile.TileContext(nc) as tc:
        _emit(nc, tc, lg, tg, mk, mp, ft, idf, idb, zm, mb, out)
    _split_sync_waits(nc)
    return nc


def _emit(nc, tc, lg, tg, mk, mp, ft, idf, idb, zm, mb, out):
    from contextlib import ExitStack

    ctx = ExitStack()
    with ctx:
        singles = ctx.enter_context(tc.tile_pool(name="singles", bufs=1))
        lpool = ctx.enter_context(tc.tile_pool(name="lpool", bufs=3))
        qpool = ctx.enter_context(tc.tile_pool(name="qpool", bufs=3))
        mqpool = ctx.enter_context(tc.tile_pool(name="mqpool", bufs=3))
        spool = ctx.enter_context(tc.tile_pool(name="spool", bufs=2))
        ppool = ctx.enter_context(tc.tile_pool(name="ppool", bufs=1))
        fpool = ctx.enter_context(tc.tile_pool(name="fpool", bufs=1))
        scratch = ctx.enter_context(tc.tile_pool(name="scratch", bufs=1))
        tiny = ctx.enter_context(tc.tile_pool(name="tiny", bufs=1))
        cpool = ctx.enter_context(tc.tile_pool(name="cpool", bufs=2))
        pst = ctx.enter_context(tc.tile_pool(name="pst", bufs=2, space="PSUM"))
        pss = ctx.enter_context(tc.tile_pool(name="pss", bufs=1, space="PSUM"))
        psc = ctx.enter_context(tc.tile_pool(name="psc", bufs=2, space="PSUM"))
        psf = ctx.enter_context(tc.tile_pool(name="psf", bufs=1, space="PSUM"))

        # constants + accumulator
        ones = singles.tile([128, 1], F32)
        nc.vector.memset(ones, 1.0)
        acc = singles.tile([128, NPART], F32)
        nc.vector.memset(acc, 0.0)
        ident_f = singles.tile([128, 128], F32)
        nc.sync.dma_start(out=ident_f, in_=idf[:, :])
        ident_b = singles.tile([128, 128], BF16)
        nc.sync.dma_start(out=ident_b, in_=idb[:, :])
        zm_t = singles.tile([128, 128], F32)
        nc.sync.dma_start(out=zm_t, in_=zm[:, :])

        # ----------------- focal loss partials -----------------
        tg_t = singles.tile([128, BP, XB], I32)
        nc.sync.dma_start(out=tg_t, in_=tg.rearrange("b p x -> p b x"))
        tg_b = singles.tile([128, FD], BF16)
        nc.vector.tensor_copy(out=tg_b, in_=tg_t.rearrange("p b x -> p (b x)"))

        s_acc = None
        pt_acc = None
        q_prev = None
        mq_prev = None
        for c in range(C):
            l_c = lpool.tile([128, BP, XB], F32, tag="l")
            nc.sync.dma_start(out=l_c, in_=lg[:, c].rearrange("b p x -> p b x"))
            q_c = qpool.tile([128, FD], BF16, tag="q")
            nc.scalar.activation(
                out=q_c, in_=l_c.rearrange("p b x -> p (b x)"), func=AF.Exp
            )
            mq_c = mqpool.tile([128, FD], BF16, tag="mq")
            nc.vector.scalar_tensor_tensor(
                out=mq_c, in0=tg_b, scalar=float(c), in1=q_c,
                op0=OP.is_equal, op1=OP.mult,
            )
            if c == 0:
                q_prev, mq_prev = q_c, mq_c
            elif c == 1:
                s_acc = spool.tile([128, FD], BF16, tag="s")
                nc.vector.tensor_tensor(out=s_acc, in0=q_prev, in1=q_c, op=OP.add)
                pt_acc = spool.tile([128, FD], BF16, tag="pt")
                nc.vector.tensor_tensor(out=pt_acc, in0=mq_prev, in1=mq_c, op=OP.add)
                q_prev = mq_prev = None
            else:
                s_new = spool.tile([128, FD], BF16, tag="s")
                nc.vector.tensor_tensor(out=s_new, in0=s_acc, in1=q_c, op=OP.add)
                s_acc = s_new
                pt_new = spool.tile([128, FD], BF16, tag="pt")
                nc.vector.tensor_tensor(out=pt_new, in0=pt_acc, in1=mq_c, op=OP.add)
                pt_acc = pt_new

        ln_s = scratch.tile([128, FD], BF16, tag="lns")
        nc.scalar.activation(out=ln_s, in_=s_acc, func=AF.Ln)
        ln_pt = scratch.tile([128, FD], BF16, tag="lnpt")
        nc.scalar.activation(out=ln_pt, in_=pt_acc, func=AF.Ln)
        ce = scratch.tile([128, FD], BF16, tag="ce")
        nc.vector.tensor_tensor(out=ce, in0=ln_s, in1=ln_pt, op=OP.subtract)
        p_t = scratch.tile([128, FD], BF16, tag="p")
        nc.scalar.activation(out=p_t, in_=ce, func=AF.Exp, scale=-1.0)
        u_t = scratch.tile([128, FD], BF16, tag="u")
        nc.vector.tensor_scalar(
            out=u_t, in0=p_t, scalar1=-1.0, scalar2=1.0, op0=OP.mult, op1=OP.add
        )
        v_t = scratch.tile([128, FD], BF16, tag="v")
        nc.vector.tensor_tensor(out=v_t, in0=u_t, in1=u_t, op=OP.mult)
        w_t = scratch.tile([128, FD], BF16, tag="wt")
        nc.vector.tensor_tensor(out=w_t, in0=v_t, in1=ce, op=OP.mult)
        w_junk = scratch.tile([128, FD], BF16, tag="wj")
        nc.vector.tensor_scalar(
            out=w_junk, in0=w_t, scalar1=GAMMA_SCALE, scalar2=0.0,
            op0=OP.mult, op1=OP.add, accum_out=acc[:, K_FOCAL:K_FOCAL + 1],
        )

        # ----------------- consensus partials -----------------
        p_tiles = []
        for i in range(3):
            p_i = ppool.tile([128, BP, XB], F32, tag=f"mp{i}")
            nc.sync.dma_start(out=p_i, in_=mp[i].rearrange("b p x -> p b x"))
            p_tiles.append(p_i)
            sj = scratch.tile([128, FD], BF16, tag="wj")
            nc.vector.tensor_scalar(
                out=sj, in0=p_i.rearrange("p b x -> p (b x)"), scalar1=1.0,
                scalar2=0.0, op0=OP.mult, op1=OP.add,
                accum_out=acc[:, K_S + i:K_S + i + 1],
            )
        for k, (i, j) in enumerate(((0, 1), (0, 2), (1, 2))):
            ij = scratch.tile([128, FD], BF16, tag="wt")
            nc.vector.tensor_tensor(
                out=ij, in0=p_tiles[i].rearrange("p b x -> p (b x)"),
                in1=p_tiles[j].rearrange("p b x -> p (b x)"), op=OP.mult,
            )
            ij2 = scratch.tile([128, FD], BF16, tag="wj")
            nc.vector.tensor_scalar(
                out=ij2, in0=ij, scalar1=1.0, scalar2=0.0,
                op0=OP.mult, op1=OP.add,
                accum_out=acc[:, K_I + k:K_I + k + 1],
            )

        # ----------------- circularity partials -----------------
        m_t = singles.tile([128, BP, 2, 256], F32)
        nc.sync.dma_start(out=m_t, in_=mk.rearrange("b c p w -> p b c w"))
        for b in range(BP):
            ps_b = psc.tile([128, 2, 256], F32, tag="circ")
            nc.tensor.matmul(
                out=ps_b, lhsT=zm_t, rhs=m_t[:, b], start=True, stop=True
            )
            nc.vector.tensor_reduce(
                out=acc[:, K_EX + b:K_EX + b + 1], in_=ps_b,
                axis=AX.XY, op=OP.add, apply_absolute_value=True,
            )
            aj = scratch.tile([128, XB], BF16, tag="actj")
            nc.scalar.activation(
                out=aj, in_=m_t[:, b].rearrange("p c w -> p (c w)"), func=AF.Copy,
                accum_out=acc[:, K_AREA + b:K_AREA + b + 1],
            )
        mb_t = singles.tile([1, BP, 2, 256], F32)
        nc.sync.dma_start(out=mb_t, in_=mb[:, :, :, :])
        d_bnd = cpool.tile([1, BP, 256], BF16, tag="dbnd")
        nc.vector.tensor_tensor(
            out=d_bnd, in0=mb_t[:, :, 1], in1=mb_t[:, :, 0], op=OP.subtract
        )
        nc.vector.tensor_reduce(
            out=acc[0:1, K_EXB:K_EXB + BP], in_=d_bnd,
            axis=AX.X, op=OP.add, apply_absolute_value=True,
        )
        d_y = singles.tile([128, BP, 2, 255], BF16)
        nc.vector.tensor_tensor(
            out=d_y, in0=m_t[:, :, :, 1:256], in1=m_t[:, :, :, 0:255],
            op=OP.subtract,
        )
        nc.vector.tensor_reduce(
            out=acc[:, K_EY:K_EY + 8].rearrange("p (b c) -> p b c", b=BP),
            in_=d_y, axis=AX.X, op=OP.add, apply_absolute_value=True,
        )

        # ----------------- contrastive partials -----------------
        f_t = fpool.tile([128, 8, DF], F32)
        nc.sync.dma_start(out=f_t, in_=ft.rearrange("k p d -> p k d"))
        ss = tiny.tile([128, 8], F32, tag="ss")
        for k in range(8):
            fsq = scratch.tile([128, DF], BF16, tag="actj")
            nc.scalar.activation(
                out=fsq, in_=f_t[:, k], func=AF.Square,
                accum_out=ss[:, k:k + 1],
            )
        # rsqrt via exp(-0.5*ln(ss)) (stays in the exp/ln table set),
        # then one Newton step y' = y*(1.5 - 0.5*ss*y^2)
        lns_t = tiny.tile([128, 8], F32, tag="lnss")
        nc.scalar.activation(out=lns_t, in_=ss, func=AF.Ln)
        inv0 = tiny.tile([128, 8], F32, tag="inv0")
        nc.scalar.activation(out=inv0, in_=lns_t, func=AF.Exp, scale=-0.5)
        t1 = tiny.tile([128, 8], F32, tag="t1")
        nc.vector.tensor_tensor(out=t1, in0=inv0, in1=inv0, op=OP.mult)
        t2 = tiny.tile([128, 8], F32, tag="t2")
        nc.vector.tensor_tensor(out=t2, in0=t1, in1=ss, op=OP.mult)
        t3 = tiny.tile([128, 8], F32, tag="t3")
        nc.vector.tensor_scalar(
            out=t3, in0=t2, scalar1=-0.5, scalar2=1.5, op0=OP.mult, op1=OP.add
        )
        inv = tiny.tile([128, 8], F32, tag="inv")
        nc.vector.tensor_tensor(out=inv, in0=inv0, in1=t3, op=OP.mult)

        fn = fpool.tile([128, 8, DF], BF16)
        for k in range(8):
            nc.vector.tensor_scalar(
                out=fn[:, k], in0=f_t[:, k], scalar1=inv[:, k:k + 1],
                scalar2=None, op0=OP.mult,
            )
        ftr = [
            fpool.tile([128, 8, 128], BF16, tag=f"ftr{dc}", name=f"ftr{dc}")
            for dc in range(4)
        ]
        for k in range(8):
            for dc in range(4):
                ps_t = pst.tile([128, 128], BF16, tag="tr")
                nc.tensor.transpose(
                    out=ps_t, in_=fn[:, k, dc * 128:(dc + 1) * 128],
                    identity=ident_b,
                )
                nc.vector.tensor_copy(out=ftr[dc][:, k], in_=ps_t)
        sim = []
        for half in range(2):
            ps_h = pss.tile([128, 512], F32, tag=f"sim{half}")
            for dc in range(4):
                nc.tensor.matmul(
                    out=ps_h,
                    lhsT=ftr[dc][:, 0],
                    rhs=ftr[dc].rearrange("p k x -> p (k x)")[
                        :, half * 512:(half + 1) * 512],
                    start=(dc == 0), stop=(dc == 3),
                )
            sim.append(ps_h)
        # rolled features: diagonal = local column block 0, positive = block 4
        nc.vector.scalar_tensor_tensor(
            out=sim[0][:, 0:128], in0=ident_f, scalar=-1e4,
            in1=sim[0][:, 0:128], op0=OP.mult, op1=OP.add,
        )
        rsum = tiny.tile([128, 2], F32, tag="rsum")
        for half in range(2):
            e_h = scratch.tile([128, 512], BF16, tag="actj")
            nc.scalar.activation(
                out=e_h, in_=sim[half], func=AF.Exp, scale=1.0 / TEMP,
                accum_out=rsum[:, half:half + 1],
            )
        rtot = tiny.tile([128, 1], F32, tag="rtot")
        nc.vector.tensor_tensor(
            out=rtot, in0=rsum[:, 0:1], in1=rsum[:, 1:2], op=OP.add
        )
        lse = tiny.tile([128, 1], F32, tag="lse")
        nc.scalar.activation(out=lse, in_=rtot, func=AF.Ln)
        posj = scratch.tile([128, 128], F32, tag="posj")
        pos = tiny.tile([128, 1], F32, tag="pos")
        nc.vector.tensor_tensor(
            out=posj, in0=sim[1][:, 0:128], in1=ident_f, op=OP.mult
        )
        posj2 = scratch.tile([128, 128], BF16, tag="posj2")
        nc.vector.tensor_scalar(
            out=posj2, in0=posj, scalar1=1.0 / TEMP, scalar2=0.0,
            op0=OP.mult, op1=OP.add, accum_out=pos,
        )
        nc.vector.tensor_tensor(
            out=acc[:, K_CONTRAST:K_CONTRAST + 1], in0=lse, in1=pos,
            op=OP.subtract,
        )

        # ----------------- partition-reduce + store -----------------
        pfin = psf.tile([1, NPART], F32)
        nc.tensor.matmul(out=pfin, lhsT=ones, rhs=acc, start=True, stop=True)
        out_t = tiny.tile([1, NPART], F32, tag="outt")
        nc.vector.tensor_copy(out=out_t, in_=pfin)
        nc.sync.dma_start(out=out[:, :], in_=out_t)


def _zmat():
    ident = np.eye(128, dtype=np.float32)
    z = np.roll(ident, -1, axis=0) - ident
    z[:, 127] = 0.0
    return np.ascontiguousarray(z)


def _host_inputs(logits, target, features, masks, method_preds):
    """Slice/reshape full inputs into per-core input maps."""
    ident = np.eye(128, dtype=np.float32)
    consts = {
        "idf": ident,
        "idb": ident.astype(ml_dtypes.bfloat16),
        # zm = P127 @ (Cyc - I): row-diff matrix with output row 127 zeroed
        "zm": _zmat(),
    }
    in_maps = []
    for c in range(NCORES):
        b0 = c * BP
        in_maps.append({
            "lg": np.ascontiguousarray(
                logits[b0:b0 + BP].reshape(BP, C, 128, XB)),
            "tg": np.ascontiguousarray(
                target[b0:b0 + BP].reshape(BP, 128, XB)),
            "mk": np.ascontiguousarray(
                masks[b0:b0 + BP, 0].reshape(BP, 2, 128, 256)),
            "mp": np.ascontiguousarray(
                method_preds[:, b0:b0 + BP].reshape(3, BP, 128, XB)),
            "mb": np.ascontiguousarray(
                masks[b0:b0 + BP, 0, 127:129, :].reshape(1, BP, 2, 256)),
            "ft": np.ascontiguousarray(
                np.roll(features, -c * 128, axis=0).reshape(8, 128, DF)),
            **consts,
        })
    return in_maps


def _combine(partials):
    """Host-side combination of the per-core [1,32] partial vectors."""
    P = np.stack([np.asarray(p).reshape(-1).astype(np.float64)
                  for p in partials])  # [8,32]
    focal = P[:, K_FOCAL].sum() / (B * HW)
    contrast = 0.5 * P[:, K_CONTRAST].sum() / BF

    circ_total = 0.0
    for c in range(NCORES):
        for b in range(BP):
            area = P[c, K_AREA + b]
            ex = P[c, K_EX + b] + P[c, K_EXB + b]
            ey = P[c, K_EY + 2 * b] + P[c, K_EY + 2 * b + 1]
            per = ex + ey
            if area > 0 and per > 0:
                circv = 4.0 * np.pi * area / max(per, 1e-12) ** 2
                circ_total += (circv - 1.0) ** 2
    circ = 0.1 * circ_total / B

    S = P[:, K_S:K_S + 3].sum(axis=0)
    I = P[:, K_I:K_I + 3].sum(axis=0)
    cons_total = 0.0
    for k, (i, j) in enumerate(((0, 1), (0, 2), (1, 2))):
        union = S[i] + S[j] - I[k]
        iou = I[k] / (union + 1e-6)
        cons_total += max(0.6 - iou, 0.0)
    consensus = 0.3 * cons_total / 3.0

    return np.float32(focal + contrast + circ + consensus)


_CACHED_NC = None


def _get_nc():
    global _CACHED_NC
    if _CACHED_NC is None:
        _CACHED_NC = _build_nc()
    return _CACHED_NC


def kernel(logits, target, features, masks, method_preds):
    logits = np.asarray(logits, dtype=np.float32)
    target = np.asarray(target, dtype=np.int32)
    features = np.asarray(features, dtype=np.float32)
    masks = np.asarray(masks, dtype=np.float32)
    method_preds = np.asarray(method_preds, dtype=np.float32)

    in_maps = _host_inputs(logits, target, features, masks, method_preds)
    res = run_bass_kernel_spmd(_get_nc(), in_maps, list(range(NCORES)))
    partials = [res.results[c]["partials"] for c in range(NCORES)]
    return _combine(partials)



# revision 16
# speedup vs baseline: 1.7650x; 1.7650x over previous
"""Trainium2 Bass kernel for CombinedAdvancedLoss (focal + contrastive +
circularity + consensus), data-parallel over 8 NeuronCores.

v2 design (vs baseline):
- All bulk inputs shipped bf16, partition-major contiguous (halves DMA).
- Host gathers the target-logit plane xt = logits[b, target] (pure indexing),
  so focal needs no on-chip one-hot select: ce = ln(sum_c exp(l_c)) - xt.
- Channel-sum of exp(logits) done on the Tensor engine via PSUM-accumulated
  identity matmuls (8 accumulating matmuls per 512-col chunk).
- Features shipped pre-transposed (and rolled by -core*128 rows); similarity
  computed as an unnormalized Gram matrix on the PE, then scaled by
  row/col inverse norms computed on-chip (no on-chip transposes of fn).
- Big free-axis sums use fused tensor_tensor_reduce / activation accum_out,
  spread across Vector, Scalar, GpSimd and PE so no single engine dominates.

Each core emits a [1,32] vector of linear partial sums; the host combines
them (the only nonlinear cross-core math - IoU ratios and the circularity
formula - acts on a handful of scalars).
"""

import sys

for _p in ("/opt/trn_rl_repo",):
    if _p not in sys.path:
        sys.path.insert(0, _p)

import numpy as np
import ml_dtypes

import concourse.bass as bass
import concourse.tile as tile
from concourse import mybir
from concourse.bass_utils import run_bass_kernel_spmd

import bass_rust as _bass_rust

# ---------------------------------------------------------------------------
# The walrus build in this container rejects >2 sync waits per instruction.
# Post-pass: hoist excess waits onto inserted same-engine NoOps.
_WAIT_CAP = 1


def _split_sync_waits(nc):
    n = 0
    for fn in nc.m.functions:
        for blk in fn.blocks:
            insts = blk.instructions
            i = 0
            while i < len(insts):
                inst = insts[i]
                si = inst.sync_info
                if si is not None and len(si.on_wait) > _WAIT_CAP:
                    waits = list(si.on_wait)
                    keep = waits[-_WAIT_CAP:]
                    extra = waits[:-_WAIT_CAP]
                    nops = []
                    for j in range(0, len(extra), _WAIT_CAP):
                        nop = mybir.InstNoOp(
                            name=f"I-wsplit-{n}", engine=inst.engine)
                        n += 1
                        nop.sync_info = _bass_rust.SyncInfo(
                            on_wait=extra[j:j + _WAIT_CAP], on_update=[])
                        nops.append(nop)
                    inst.sync_info = _bass_rust.SyncInfo(
                        on_wait=keep, on_update=list(si.on_update))
                    for k, nop in enumerate(nops):
                        insts.insert(i + k, nop)
                    i += len(nops)
                i += 1
# ---------------------------------------------------------------------------

F32 = mybir.dt.float32
BF16 = mybir.dt.bfloat16
AF = mybir.ActivationFunctionType
OP = mybir.AluOpType
AX = mybir.AxisListType

NCORES = 8
B, C, H, W = 32, 8, 256, 256
BP = B // NCORES          # batch items per core (4)
FD = 2048                 # free dim of a full-core pixel tile (BP*512)
DF = 512                  # feature dim
TEMP = 0.07
NPART = 32                # width of the per-core partials vector

# partials vector layout
K_FOCAL = 0               # sum 0.25*(1-p)^2 * ce
K_CONTRAST = 1            # sum (lse - pos) over this core's 128 rows
K_AREA = 2                # 4 cols: per-b mask area
K_EX = 6                  # 4 cols: per-b sum |dm/dh| (incl. half boundary)
K_EY = 10                 # 8 cols: per-(b, half) sum |dm/dw|
K_S = 18                  # 3 cols: per-method sum of preds
K_I = 21                  # 3 cols: per-pair sum pi*pj  (01, 02, 12)


def _build_nc():
    nc = bass.Bass()

    lg = nc.declare_dram_parameter("lg", [128, C, BP, 512], BF16, isOutput=False)
    xt = nc.declare_dram_parameter("xt", [128, BP, 512], BF16, isOutput=False)
    mp = nc.declare_dram_parameter("mp", [128, 3, BP, 512], BF16, isOutput=False)
    mk = nc.declare_dram_parameter("mk", [128, BP, 2, 256], BF16, isOutput=False)
    ft = nc.declare_dram_parameter("ft", [128, 4, 1024], BF16, isOutput=False)
    idf = nc.declare_dram_parameter("idf", [128, 128], F32, isOutput=False)
    idb = nc.declare_dram_parameter("idb", [128, 128], BF16, isOutput=False)
    zm3 = nc.declare_dram_parameter("zm3", [128, 3, 128], BF16, isOutput=False)
    out = nc.declare_dram_parameter("partials", [1, NPART], F32, isOutput=True)

    with tile.TileContext(nc) as tc:
        _emit(nc, tc, lg, xt, mp, mk, ft, idf, idb, zm3, out)
    _split_sync_waits(nc)
    return nc


def _emit(nc, tc, lg, xt, mp, mk, ft, idf, idb, zm3, out):
    from contextlib import ExitStack

    ctx = ExitStack()
    with ctx:
        singles = ctx.enter_context(tc.tile_pool(name="singles", bufs=1))
        scratch = ctx.enter_context(tc.tile_pool(name="scratch", bufs=2))
        tiny = ctx.enter_context(tc.tile_pool(name="tiny", bufs=1))
        psA = ctx.enter_context(tc.tile_pool(name="psA", bufs=1, space="PSUM"))
        psB = ctx.enter_context(tc.tile_pool(name="psB", bufs=2, space="PSUM"))
        psC = ctx.enter_context(tc.tile_pool(name="psC", bufs=1, space="PSUM"))

        # ---------------- constants + accumulator ----------------
        ident_f = singles.tile([128, 128], F32)
        nc.sync.dma_start(out=ident_f, in_=idf[:, :])
        ident_b = singles.tile([128, 128], BF16)
        nc.sync.dma_start(out=ident_b, in_=idb[:, :])
        zm_t = singles.tile([128, 3, 128], BF16)
        nc.sync.dma_start(out=zm_t, in_=zm3[:, :, :])
        ones_b = singles.tile([128, 1], BF16)
        nc.vector.memset(ones_b, 1.0)
        ones_f = singles.tile([128, 1], F32)
        nc.vector.memset(ones_f, 1.0)
        acc = singles.tile([128, NPART], F32)
        nc.vector.memset(acc, 0.0)

        # ---------------- input DMAs ----------------
        lg_t = singles.tile([128, C, BP, 512], BF16)
        for h in range(4):
            nc.sync.dma_start(
                out=lg_t[:, 2 * h:2 * h + 2], in_=lg[:, 2 * h:2 * h + 2])
        ft_t = singles.tile([128, 4, 1024], BF16)
        nc.sync.dma_start(out=ft_t, in_=ft[:, :, :])
        xt_t = singles.tile([128, BP, 512], BF16)
        nc.sync.dma_start(out=xt_t, in_=xt[:, :, :])
        mp_t = singles.tile([128, 3, BP, 512], BF16)
        nc.sync.dma_start(out=mp_t, in_=mp[:, :, :, :])
        mk_t = singles.tile([128, BP, 2, 256], BF16)
        nc.sync.dma_start(out=mk_t, in_=mk[:, :, :, :])

        # ---------------- contrastive part 1: norms ----------------
        # sq = ftT**2 ; ss[j] = sum_d ftT[d, j]^2 via PE column sums
        sq = singles.tile([128, 4, 1024], BF16)
        nc.vector.tensor_tensor(
            out=sq.rearrange("p a b -> p (a b)"),
            in0=ft_t.rearrange("p a b -> p (a b)"),
            in1=ft_t.rearrange("p a b -> p (a b)"), op=OP.mult)
        ss_ps = psC.tile([1, 1024], F32, tag="g", name="ss_ps")
        for half in range(2):
            for dc in range(4):
                nc.tensor.matmul(
                    out=ss_ps[:, 512 * half:512 * (half + 1)],
                    lhsT=ones_b,
                    rhs=sq[:, dc, 512 * half:512 * (half + 1)],
                    start=(dc == 0), stop=(dc == 3))
        # colinv = 1/sqrt(ss) = exp(-0.5*ln(ss)) laid out along free axis
        lnss = tiny.tile([1, 1024], F32, tag="lnss")
        nc.scalar.activation(out=lnss, in_=ss_ps, func=AF.Ln)
        colinv = tiny.tile([1, 1024], F32, tag="colinv")
        nc.scalar.activation(out=colinv, in_=lnss, func=AF.Exp, scale=-0.5)
        # broadcast colinv down partitions: ones[1,128].T @ colinv[1,1024]
        ones_r = singles.tile([1, 128], F32)
        nc.vector.memset(ones_r, 1.0)
        cb_ps = psC.tile([128, 1024], F32, tag="g", name="cb_ps")
        for half in range(2):
            nc.tensor.matmul(
                out=cb_ps[:, 512 * half:512 * (half + 1)],
                lhsT=ones_r,
                rhs=colinv[:, 512 * half:512 * (half + 1)],
                start=True, stop=True)
        colbc = singles.tile([128, 1024], F32)
        nc.vector.tensor_copy(out=colbc, in_=cb_ps)
        # rowinv/T for the exp scale (local row r = local col r after roll)
        rT_ps = psB.tile([128, 1], F32, tag="sm", name="rT_ps")
        nc.tensor.transpose(
            out=rT_ps, in_=colinv[:, 0:128], identity=ident_f[0:1, 0:1])
        rowinv = tiny.tile([128, 1], F32, tag="rowinv")
        nc.vector.tensor_scalar(
            out=rowinv, in0=rT_ps, scalar1=1.0 / TEMP, scalar2=None,
            op0=OP.mult)

        # ---------------- contrastive part 2: Gram + lse ----------------
        g_ps = psC.tile([128, 1024], F32, tag="g")
        for half in range(2):
            for dc in range(4):
                nc.tensor.matmul(
                    out=g_ps[:, 512 * half:512 * (half + 1)],
                    lhsT=ft_t[:, dc, 0:128],
                    rhs=ft_t[:, dc, 512 * half:512 * (half + 1)],
                    start=(dc == 0), stop=(dc == 3))
        st2 = singles.tile([128, 1024], F32)
        nc.vector.tensor_tensor(out=st2, in0=g_ps, in1=colbc, op=OP.mult)
        # mask own diagonal (local column block 0)
        nc.vector.scalar_tensor_tensor(
            out=st2[:, 0:128], in0=ident_f, scalar=-1e5,
            in1=st2[:, 0:128], op0=OP.mult, op1=OP.add)
        esim = scratch.tile([128, 1024], BF16, tag="big")
        rsum = tiny.tile([128, 1], F32, tag="rsum")
        nc.scalar.activation(
            out=esim, in_=st2, func=AF.Exp, scale=rowinv, accum_out=rsum)
        # pos = st2[r, 512+r] * rowinv (sim of the positive pair / T)
        posj = scratch.tile([128, 128], F32, tag="posj")
        posr = tiny.tile([128, 1], F32, tag="posr")
        nc.vector.scalar_tensor_tensor(
            out=posj, in0=st2[:, 512:640], scalar=1.0, in1=ident_f,
            op0=OP.mult, op1=OP.mult, accum_out=posr)
        lse = tiny.tile([128, 1], F32, tag="lse")
        nc.scalar.activation(out=lse, in_=rsum, func=AF.Ln)
        post = tiny.tile([128, 1], F32, tag="post")
        nc.vector.tensor_scalar(
            out=post, in0=posr, scalar1=rowinv, scalar2=None, op0=OP.mult)
        nc.vector.tensor_tensor(
            out=acc[:, K_CONTRAST:K_CONTRAST + 1], in0=lse, in1=post,
            op=OP.subtract)

        # ---------------- focal ----------------
        q = singles.tile([128, C, 2048], BF16, name="q")
        for h in range(4):
            nc.scalar.activation(
                out=q[:, 2 * h:2 * h + 2].rearrange("p c x -> p (c x)"),
                in_=lg_t[:, 2 * h:2 * h + 2].rearrange("p c b x -> p (c b x)"),
                func=AF.Exp)
        s_ps = psA.tile([128, 2048], F32, tag="s")
        for j in range(4):
            for c in range(C):
                nc.tensor.matmul(
                    out=s_ps[:, 512 * j:512 * (j + 1)],
                    lhsT=ident_b,
                    rhs=q[:, c, 512 * j:512 * (j + 1)],
                    start=(c == 0), stop=(c == C - 1))
        ln_s = singles.tile([128, 2048], BF16)
        nc.scalar.activation(out=ln_s, in_=s_ps, func=AF.Ln)
        ce = singles.tile([128, 2048], BF16)
        nc.vector.tensor_tensor(
            out=ce, in0=ln_s, in1=xt_t.rearrange("p b x -> p (b x)"),
            op=OP.subtract)
        p_t = singles.tile([128, 2048], BF16)
        nc.scalar.activation(out=p_t, in_=ce, func=AF.Exp, scale=-1.0)
        u_t = singles.tile([128, 2048], BF16)
        nc.vector.tensor_scalar(
            out=u_t, in0=p_t, scalar1=-1.0, scalar2=1.0,
            op0=OP.mult, op1=OP.add)
        v_t = singles.tile([128, 2048], BF16)
        nc.vector.tensor_tensor(out=v_t, in0=u_t, in1=u_t, op=OP.mult)
        wj = scratch.tile([128, 2048], BF16, tag="wj")
        nc.vector.scalar_tensor_tensor(
            out=wj, in0=v_t, scalar=0.25, in1=ce, op0=OP.mult, op1=OP.mult,
            accum_out=acc[:, K_FOCAL:K_FOCAL + 1])

        # ---------------- circularity ----------------
        # ex (row diffs incl. cross-half boundary) via 3 matmuls per image
        for b in range(BP):
            cps = psB.tile([128, 512], F32, tag="sm", name=f"cps{b}")
            nc.tensor.matmul(
                out=cps[:, 0:256], lhsT=zm_t[:, 0], rhs=mk_t[:, b, 0],
                start=True, stop=False)
            nc.tensor.matmul(
                out=cps[:, 0:256], lhsT=zm_t[:, 1], rhs=mk_t[:, b, 1],
                start=False, stop=True)
            nc.tensor.matmul(
                out=cps[:, 256:512], lhsT=zm_t[:, 2], rhs=mk_t[:, b, 1],
                start=True, stop=True)
            if b < 2:
                nc.vector.tensor_reduce(
                    out=acc[:, K_EX + b:K_EX + b + 1], in_=cps,
                    axis=AX.XY, op=OP.add, apply_absolute_value=True)
            else:
                absj = scratch.tile([128, 512], BF16, tag="aj", name=f"ab{b}")
                nc.scalar.activation(
                    out=absj, in_=cps, func=AF.Abs,
                    accum_out=acc[:, K_EX + b:K_EX + b + 1])
        # area sums: 2 on scalar, 2 on vector
        for b in range(BP):
            aj = scratch.tile([128, 512], BF16, tag="aj")
            if b < 2:
                nc.scalar.activation(
                    out=aj, in_=mk_t[:, b].rearrange("p h w -> p (h w)"),
                    func=AF.Copy,
                    accum_out=acc[:, K_AREA + b:K_AREA + b + 1])
            else:
                nc.vector.tensor_scalar(
                    out=aj, in0=mk_t[:, b].rearrange("p h w -> p (h w)"),
                    scalar1=1.0, scalar2=0.0, op0=OP.mult, op1=OP.add,
                    accum_out=acc[:, K_AREA + b:K_AREA + b + 1])
        # ey: in-row diffs
        d_y = singles.tile([128, BP, 2, 255], BF16)
        nc.gpsimd.tensor_tensor(
            out=d_y, in0=mk_t[:, :, :, 1:256], in1=mk_t[:, :, :, 0:255],
            op=OP.subtract)
        nc.vector.tensor_reduce(
            out=acc[:, K_EY:K_EY + 8].rearrange("p (b c) -> p b c", b=BP),
            in_=d_y, axis=AX.X, op=OP.add, apply_absolute_value=True)

        # ---------------- consensus ----------------
        # S_i column-partials via PE: sum_p mp[p, 128k+a] lands in sps[a, i];
        # the final ones.T @ acc matmul closes the partition sum.
        sps = psB.tile([128, 3], F32, tag="sm", name="sps")
        for i in range(3):
            mflat = mp_t[:, i].rearrange("p b x -> p (b x)")
            for k in range(16):
                nc.tensor.matmul(
                    out=sps[:, i:i + 1],
                    lhsT=mflat[:, 128 * k:128 * (k + 1)],
                    rhs=ones_b,
                    start=(k == 0), stop=(k == 15))
        nc.vector.tensor_copy(out=acc[:, K_S:K_S + 3], in_=sps)
        for k, (i, j) in enumerate(((0, 1), (0, 2), (1, 2))):
            ij = scratch.tile([128, 2048], BF16, tag="wj", name=f"ij{k}")
            nc.vector.scalar_tensor_tensor(
                out=ij, in0=mp_t[:, i].rearrange("p b x -> p (b x)"),
                scalar=1.0, in1=mp_t[:, j].rearrange("p b x -> p (b x)"),
                op0=OP.mult, op1=OP.mult,
                accum_out=acc[:, K_I + k:K_I + k + 1])

        # ---------------- partition-reduce + store ----------------
        pfin = psB.tile([1, NPART], F32, tag="sm", name="pfin")
        nc.tensor.matmul(out=pfin, lhsT=ones_f, rhs=acc, start=True, stop=True)
        out_t = tiny.tile([1, NPART], F32, tag="outt")
        nc.vector.tensor_copy(out=out_t, in_=pfin)
        nc.sync.dma_start(out=out[:, :], in_=out_t)


def _zmats():
    """lhsT matrices for row-diff matmuls: out[r] = sum_p Z[p, r] * m[p]."""
    zmA = np.zeros((128, 128), dtype=np.float32)
    zmB = np.zeros((128, 128), dtype=np.float32)
    zmC = np.zeros((128, 128), dtype=np.float32)
    for r in range(127):
        zmA[r + 1, r] = 1.0
        zmA[r, r] = -1.0
        zmC[r + 1, r] = 1.0
        zmC[r, r] = -1.0
    zmA[127, 127] = -1.0   # half0 row127: -m0[127], completed by zmB
    zmB[0, 127] = 1.0      # + m1[0]  -> cross-half boundary diff
    return np.stack([zmA, zmB, zmC], axis=1)  # [128, 3, 128]


def _host_inputs(logits, target, features, masks, method_preds):
    """Slice/reshape/convert full inputs into per-core input maps."""
    bf = ml_dtypes.bfloat16
    ident = np.eye(128, dtype=np.float32)
    consts = {
        "idf": ident,
        "idb": ident.astype(bf),
        "zm3": np.ascontiguousarray(_zmats().astype(bf)),
    }
    # gather target logit plane on host (pure indexing)
    xt_full = np.take_along_axis(
        logits, target[:, None].astype(np.int64), axis=1)[:, 0]  # [B, H, W]
    in_maps = []
    for c in range(NCORES):
        b0 = c * BP
        lgs = logits[b0:b0 + BP]                                # [4,8,256,256]
        lg_pm = lgs.reshape(BP, C, 128, 512).transpose(2, 1, 0, 3)
        xt_pm = xt_full[b0:b0 + BP].reshape(BP, 128, 512).transpose(1, 0, 2)
        mp_pm = method_preds[:, b0:b0 + BP].reshape(
            3, BP, 128, 512).transpose(2, 0, 1, 3)
        mk_pm = masks[b0:b0 + BP, 0].reshape(BP, 2, 128, 256).transpose(
            2, 0, 1, 3)
        fr = np.roll(features, -c * 128, axis=0)                # [1024, 512]
        ft_pm = fr.T.reshape(4, 128, 1024).transpose(1, 0, 2)
        in_maps.append({
            "lg": np.ascontiguousarray(lg_pm.astype(bf)),
            "xt": np.ascontiguousarray(xt_pm.astype(bf)),
            "mp": np.ascontiguousarray(mp_pm.astype(bf)),
            "mk": np.ascontiguousarray(mk_pm.astype(bf)),
            "ft": np.ascontiguousarray(ft_pm.astype(bf)),
            **consts,
        })
    return in_maps


def _combine(partials):
    """Host-side combination of the per-core [1,32] partial vectors."""
    P = np.stack([np.asarray(p).reshape(-1).astype(np.float64)
                  for p in partials])  # [8,32]
    HW = H * W
    focal = P[:, K_FOCAL].sum() / (B * HW)
    contrast = 0.5 * P[:, K_CONTRAST].sum() / 1024

    circ_total = 0.0
    for c in range(NCORES):
        for b in range(BP):
            area = P[c, K_AREA + b]
            ex = P[c, K_EX + b]
            ey = P[c, K_EY + 2 * b] + P[c, K_EY + 2 * b + 1]
            per = ex + ey
            if area > 0 and per > 0:
                circv = 4.0 * np.pi * area / max(per, 1e-12) ** 2
                circ_total += (circv - 1.0) ** 2
    circ = 0.1 * circ_total / B

    S = P[:, K_S:K_S + 3].sum(axis=0)
    I = P[:, K_I:K_I + 3].sum(axis=0)
    cons_total = 0.0
    for k, (i, j) in enumerate(((0, 1), (0, 2), (1, 2))):
        union = S[i] + S[j] - I[k]
        iou = I[k] / (union + 1e-6)
        cons_total += max(0.6 - iou, 0.0)
    consensus = 0.3 * cons_total / 3.0

    return np.float32(focal + contrast + circ + consensus)


_CACHED_NC = None


def _get_nc():
    global _CACHED_NC
    if _CACHED_NC is None:
        _CACHED_NC = _build_nc()
    return _CACHED_NC


def kernel(logits, target, features, masks, method_preds):
    logits = np.asarray(logits, dtype=np.float32)
    target = np.asarray(target, dtype=np.int32)
    features = np.asarray(features, dtype=np.float32)
    masks = np.asarray(masks, dtype=np.float32)
    method_preds = np.asarray(method_preds, dtype=np.float32)

    in_maps = _host_inputs(logits, target, features, masks, method_preds)
    res = run_bass_kernel_spmd(_get_nc(), in_maps, list(range(NCORES)))
    partials = [res.results[c]["partials"] for c in range(NCORES)]
    return _combine(partials)


# revision 18
# speedup vs baseline: 1.9341x; 1.0958x over previous
"""Trainium2 Bass kernel for CombinedAdvancedLoss (focal + contrastive +
circularity + consensus), data-parallel over 8 NeuronCores.

v3 design:
- logits shipped fp8-e4m3 (2MB/core), everything else bf16, all
  partition-major contiguous. Host gathers the target-logit plane xt
  (pure indexing), so focal is ce = ln(sum_c exp(l_c)) - xt.
- Focal pipelined per image: exp chunk -> 8 accumulating identity matmuls
  into PSUM -> ln -> elementwise tail (2 half-chunks).
- Features shipped pre-transposed+rolled; similarity = unnormalized Gram
  (PE) scaled post-hoc by on-chip row/col inverse norms.
- Work spread across engines: scalar (exp/ln), vector (fused
  scalar_tensor_tensor product-sums, reduces), PE (channel sums, row-diff
  matmuls, column sums), gpsimd (feature squares, row diffs).
- Two DMA queues (SP + Activation) so small inputs land while logits
  stream.

Each core emits a [1,32] vector of linear partial sums; the host combines
them (IoU ratios and the circularity formula act on a handful of scalars).
"""

import sys

for _p in ("/opt/trn_rl_repo",):
    if _p not in sys.path:
        sys.path.insert(0, _p)

import numpy as np
import ml_dtypes

import concourse.bass as bass
import concourse.tile as tile
from concourse import mybir
from concourse.bass_utils import run_bass_kernel_spmd

import bass_rust as _bass_rust

# ---------------------------------------------------------------------------
# The walrus build in this container rejects >2 sync waits per instruction.
# Post-pass: hoist excess waits onto inserted same-engine NoOps.
_WAIT_CAP = 1


def _split_sync_waits(nc):
    n = 0
    for fn in nc.m.functions:
        for blk in fn.blocks:
            insts = blk.instructions
            i = 0
            while i < len(insts):
                inst = insts[i]
                si = inst.sync_info
                if si is not None and len(si.on_wait) > _WAIT_CAP:
                    waits = list(si.on_wait)
                    keep = waits[-_WAIT_CAP:]
                    extra = waits[:-_WAIT_CAP]
                    nops = []
                    for j in range(0, len(extra), _WAIT_CAP):
                        nop = mybir.InstNoOp(
                            name=f"I-wsplit-{n}", engine=inst.engine)
                        n += 1
                        nop.sync_info = _bass_rust.SyncInfo(
                            on_wait=extra[j:j + _WAIT_CAP], on_update=[])
                        nops.append(nop)
                    inst.sync_info = _bass_rust.SyncInfo(
                        on_wait=keep, on_update=list(si.on_update))
                    for k, nop in enumerate(nops):
                        insts.insert(i + k, nop)
                    i += len(nops)
                i += 1
# ---------------------------------------------------------------------------

F32 = mybir.dt.float32
BF16 = mybir.dt.bfloat16
FP8 = mybir.dt.float8e4
AF = mybir.ActivationFunctionType
OP = mybir.AluOpType
AX = mybir.AxisListType

NCORES = 8
B, C, H, W = 32, 8, 256, 256
BP = B // NCORES          # batch items per core (4)
FD = 2048                 # free dim of a full-core pixel tile (BP*512)
TEMP = 0.07
NPART = 32                # width of the per-core partials vector

# partials vector layout
K_FOCAL = 0               # 2 cols (half-chunks): sum 0.25*(1-p)^2 * ce
K_CONTRAST = 2            # sum (lse - pos) over this core's 128 rows
K_AREA = 3                # 4 cols: per-b mask area
K_EX = 7                  # 4 cols: per-b sum |dm/dh| (incl. half boundary)
K_EY = 11                 # 8 cols: per-(b, half) sum |dm/dw|
K_S = 19                  # 3 cols: per-method sum of preds
K_I = 22                  # 3 cols: per-pair sum pi*pj  (01, 02, 12)


def _build_nc():
    nc = bass.Bass()

    lg = nc.declare_dram_parameter("lg", [128, BP, C, 512], FP8, isOutput=False)
    xt = nc.declare_dram_parameter("xt", [128, BP, 512], BF16, isOutput=False)
    mp = nc.declare_dram_parameter("mp", [128, 3, BP, 512], BF16, isOutput=False)
    mk = nc.declare_dram_parameter("mk", [128, BP, 2, 256], BF16, isOutput=False)
    ft = nc.declare_dram_parameter("ft", [128, 4, 1024], BF16, isOutput=False)
    cbf = nc.declare_dram_parameter("cbf", [128, 4, 128], BF16, isOutput=False)
    cf32 = nc.declare_dram_parameter("cf32", [128, 128], F32, isOutput=False)
    out = nc.declare_dram_parameter("partials", [1, NPART], F32, isOutput=True)

    with tile.TileContext(nc) as tc:
        _emit(nc, tc, lg, xt, mp, mk, ft, cbf, cf32, out)
    _split_sync_waits(nc)
    return nc


def _emit(nc, tc, lg, xt, mp, mk, ft, cbf, cf32, out):
    from contextlib import ExitStack

    ctx = ExitStack()
    with ctx:
        singles = ctx.enter_context(tc.tile_pool(name="singles", bufs=1))
        scratch = ctx.enter_context(tc.tile_pool(name="scratch", bufs=2))
        tiny = ctx.enter_context(tc.tile_pool(name="tiny", bufs=1))
        psA = ctx.enter_context(tc.tile_pool(name="psA", bufs=1, space="PSUM"))
        psB = ctx.enter_context(tc.tile_pool(name="psB", bufs=2, space="PSUM"))
        psC = ctx.enter_context(tc.tile_pool(name="psC", bufs=1, space="PSUM"))

        # ---------------- DMAs ----------------
        # SP queue: logits image-chunks (the critical scalar-exp feed)
        lg_t = singles.tile([128, BP, C, 512], FP8)
        for j in range(BP):
            nc.sync.dma_start(out=lg_t[:, j], in_=lg[:, j])
        mk_t = singles.tile([128, BP, 2, 256], BF16)
        nc.sync.dma_start(out=mk_t, in_=mk[:, :, :, :])
        mp_t = singles.tile([128, 3, BP, 512], BF16)
        nc.sync.dma_start(out=mp_t, in_=mp[:, :, :, :])
        # Activation queue: consts + contrastive inputs + xt
        cb_t = singles.tile([128, 4, 128], BF16)
        nc.scalar.dma_start(out=cb_t, in_=cbf[:, :, :])
        ft_t = singles.tile([128, 4, 1024], BF16)
        nc.scalar.dma_start(out=ft_t, in_=ft[:, :, :])
        xt_t = singles.tile([128, BP, 512], BF16)
        nc.scalar.dma_start(out=xt_t, in_=xt[:, :, :])
        ident_f = singles.tile([128, 128], F32)
        nc.scalar.dma_start(out=ident_f, in_=cf32[:, :])

        ident_b = cb_t[:, 0]

        # ---------------- vector-engine constants ----------------
        ones_b = singles.tile([128, 1], BF16)
        nc.vector.memset(ones_b, 1.0)
        ones_f = singles.tile([128, 1], F32)
        nc.vector.memset(ones_f, 1.0)
        acc = singles.tile([128, NPART], F32)
        nc.vector.memset(acc, 0.0)

        # ---------------- gpsimd: squares + row diffs ----------------
        sq = singles.tile([128, 4, 1024], BF16)
        nc.vector.tensor_tensor(
            out=sq[:, 0:2].rearrange("p a b -> p (a b)"),
            in0=ft_t[:, 0:2].rearrange("p a b -> p (a b)"),
            in1=ft_t[:, 0:2].rearrange("p a b -> p (a b)"), op=OP.mult)
        nc.gpsimd.tensor_tensor(
            out=sq[:, 2:4].rearrange("p a b -> p (a b)"),
            in0=ft_t[:, 2:4].rearrange("p a b -> p (a b)"),
            in1=ft_t[:, 2:4].rearrange("p a b -> p (a b)"), op=OP.mult)
        d_y = singles.tile([128, BP, 2, 255], BF16)
        nc.gpsimd.tensor_tensor(
            out=d_y, in0=mk_t[:, :, :, 1:256], in1=mk_t[:, :, :, 0:255],
            op=OP.subtract)

        # ---------------- scalar: exps + norms + focal logs ----------
        q = singles.tile([128, BP, C, 512], BF16)
        s_ps = psA.tile([128, 2048], F32, tag="s")

        def exp_chunk(j):
            nc.scalar.activation(
                out=q[:, j].rearrange("p c x -> p (c x)"),
                in_=lg_t[:, j].rearrange("p c x -> p (c x)"),
                func=AF.Exp)

        def smm_chunk(j):
            for c in range(C):
                nc.tensor.matmul(
                    out=s_ps[:, 512 * j:512 * (j + 1)],
                    lhsT=ident_b,
                    rhs=q[:, j, c],
                    start=(c == 0), stop=(c == C - 1))

        exp_chunk(0)
        exp_chunk(1)

        # contrastive part 1: column norms  ss[j] = sum_d ftT[d, j]^2
        ss_ps = psC.tile([1, 1024], F32, tag="g", name="ss_ps")
        for half in range(2):
            for dc in range(4):
                nc.tensor.matmul(
                    out=ss_ps[:, 512 * half:512 * (half + 1)],
                    lhsT=ones_b,
                    rhs=sq[:, dc, 512 * half:512 * (half + 1)],
                    start=(dc == 0), stop=(dc == 3))
        lnss = tiny.tile([1, 1024], F32, tag="lnss")
        nc.scalar.activation(out=lnss, in_=ss_ps, func=AF.Ln)
        colinv = tiny.tile([1, 1024], F32, tag="colinv")
        nc.scalar.activation(out=colinv, in_=lnss, func=AF.Exp, scale=-0.5)

        exp_chunk(2)
        exp_chunk(3)

        # ---------------- PE: focal channel sums + contrastive --------
        smm_chunk(0)
        smm_chunk(1)

        # broadcast colinv down partitions via ones[1,128].T @ colinv
        ones_r = singles.tile([1, 128], F32)
        nc.vector.memset(ones_r, 1.0)
        cb_ps = psC.tile([128, 1024], F32, tag="g", name="cb_ps")
        for half in range(2):
            nc.tensor.matmul(
                out=cb_ps[:, 512 * half:512 * (half + 1)],
                lhsT=ones_r,
                rhs=colinv[:, 512 * half:512 * (half + 1)],
                start=True, stop=True)
        colbc = singles.tile([128, 1024], F32)
        nc.vector.tensor_copy(out=colbc, in_=cb_ps)
        # rowinv = colinv[0:128] transposed, scaled by 1/T
        rT_ps = psB.tile([128, 1], F32, tag="sm", name="rT_ps")
        nc.tensor.transpose(
            out=rT_ps, in_=colinv[:, 0:128], identity=ident_f[0:1, 0:1])
        rowinv = tiny.tile([128, 1], F32, tag="rowinv")
        nc.vector.tensor_scalar(
            out=rowinv, in0=rT_ps, scalar1=1.0 / TEMP, scalar2=None,
            op0=OP.mult)

        # Gram matrix for this core's 128 rows x all 1024 columns
        g_ps = psC.tile([128, 1024], F32, tag="g", name="g_ps")
        for half in range(2):
            for dc in range(4):
                nc.tensor.matmul(
                    out=g_ps[:, 512 * half:512 * (half + 1)],
                    lhsT=ft_t[:, dc, 0:128],
                    rhs=ft_t[:, dc, 512 * half:512 * (half + 1)],
                    start=(dc == 0), stop=(dc == 3))

        smm_chunk(2)

        # circularity row-diff matmuls (3 per image, incl. half boundary)
        for b in range(BP):
            cps = psB.tile([128, 512], F32, tag="sm", name=f"cps{b}")
            nc.tensor.matmul(
                out=cps[:, 0:256], lhsT=cb_t[:, 1], rhs=mk_t[:, b, 0],
                start=True, stop=False)
            nc.tensor.matmul(
                out=cps[:, 0:256], lhsT=cb_t[:, 2], rhs=mk_t[:, b, 1],
                start=False, stop=True)
            nc.tensor.matmul(
                out=cps[:, 256:512], lhsT=cb_t[:, 3], rhs=mk_t[:, b, 1],
                start=True, stop=True)
            nc.vector.tensor_reduce(
                out=acc[:, K_EX + b:K_EX + b + 1], in_=cps,
                axis=AX.XY, op=OP.add, apply_absolute_value=True)

        smm_chunk(3)

        # consensus S_i column sums: ones.T @ mp chunks -> [1, 512]
        for i in range(3):
            sps = psB.tile([1, 512], F32, tag="sm", name=f"sps{i}")
            mflat = mp_t[:, i].rearrange("p b x -> p (b x)")
            for k in range(4):
                nc.tensor.matmul(
                    out=sps,
                    lhsT=ones_b,
                    rhs=mflat[:, 512 * k:512 * (k + 1)],
                    start=(k == 0), stop=(k == 3))
            sjunk = scratch.tile([1, 512], F32, tag="sjk", name=f"sjk{i}")
            nc.vector.tensor_scalar(
                out=sjunk, in0=sps, scalar1=1.0, scalar2=0.0,
                op0=OP.mult, op1=OP.add,
                accum_out=acc[0:1, K_S + i:K_S + i + 1])

        # ---------------- vector: circ/consensus/contrastive ----------
        for b in range(2, BP):
            aj = scratch.tile([128, 512], BF16, tag="aj", name=f"aj{b}")
            nc.vector.tensor_scalar(
                out=aj, in0=mk_t[:, b].rearrange("p h w -> p (h w)"),
                scalar1=1.0, scalar2=0.0, op0=OP.mult, op1=OP.add,
                accum_out=acc[:, K_AREA + b:K_AREA + b + 1])

        st2 = singles.tile([128, 1024], F32)
        nc.vector.tensor_tensor(out=st2, in0=g_ps, in1=colbc, op=OP.mult)
        nc.vector.scalar_tensor_tensor(
            out=st2[:, 0:128], in0=ident_f, scalar=-1e5,
            in1=st2[:, 0:128], op0=OP.mult, op1=OP.add)
        posj = scratch.tile([128, 128], F32, tag="posj")
        posr = tiny.tile([128, 1], F32, tag="posr")
        nc.vector.scalar_tensor_tensor(
            out=posj, in0=st2[:, 512:640], scalar=1.0, in1=ident_f,
            op0=OP.mult, op1=OP.mult, accum_out=posr)

        esim = scratch.tile([128, 1024], BF16, tag="esim")
        rsum = tiny.tile([128, 1], F32, tag="rsum")
        nc.scalar.activation(
            out=esim, in_=st2, func=AF.Exp, scale=rowinv, accum_out=rsum)
        lse = tiny.tile([128, 1], F32, tag="lse")
        nc.scalar.activation(out=lse, in_=rsum, func=AF.Ln)

        # consensus pair intersections (fused product+sum)
        for k, (i, j) in enumerate(((0, 1), (0, 2), (1, 2))):
            ij = scratch.tile([128, 2048], BF16, tag="wj", name=f"ij{k}")
            nc.vector.scalar_tensor_tensor(
                out=ij, in0=mp_t[:, i].rearrange("p b x -> p (b x)"),
                scalar=1.0, in1=mp_t[:, j].rearrange("p b x -> p (b x)"),
                op0=OP.mult, op1=OP.mult,
                accum_out=acc[:, K_I + k:K_I + k + 1])

        # ey: in-row diffs reduce
        nc.vector.tensor_reduce(
            out=acc[:, K_EY:K_EY + 8].rearrange("p (b c) -> p b c", b=BP),
            in_=d_y, axis=AX.X, op=OP.add, apply_absolute_value=True)

        # contrast partial: lse - pos*rowinv
        post = tiny.tile([128, 1], F32, tag="post")
        nc.vector.tensor_scalar(
            out=post, in0=posr, scalar1=rowinv, scalar2=None, op0=OP.mult)
        nc.vector.tensor_tensor(
            out=acc[:, K_CONTRAST:K_CONTRAST + 1], in0=lse, in1=post,
            op=OP.subtract)

        # ---------------- scalar: areas + focal logs ------------------
        for b in range(2):
            aj = scratch.tile([128, 512], BF16, tag="aj", name=f"ajs{b}")
            nc.scalar.activation(
                out=aj, in_=mk_t[:, b].rearrange("p h w -> p (h w)"),
                func=AF.Copy,
                accum_out=acc[:, K_AREA + b:K_AREA + b + 1])

        # ---------------- focal tail (2 half-chunks) ------------------
        ln_s = singles.tile([128, 2048], BF16)
        p_t = singles.tile([128, 2048], BF16)
        ce = singles.tile([128, 2048], BF16)
        u_t = singles.tile([128, 2048], BF16)
        v_t = singles.tile([128, 2048], BF16)
        xtf = xt_t.rearrange("p b x -> p (b x)")
        for h in range(2):
            sl = slice(1024 * h, 1024 * (h + 1))
            nc.scalar.activation(out=ln_s[:, sl], in_=s_ps[:, sl], func=AF.Ln)
        for h in range(2):
            sl = slice(1024 * h, 1024 * (h + 1))
            nc.vector.tensor_tensor(
                out=ce[:, sl], in0=ln_s[:, sl], in1=xtf[:, sl],
                op=OP.subtract)
        for h in range(2):
            sl = slice(1024 * h, 1024 * (h + 1))
            nc.scalar.activation(
                out=p_t[:, sl], in_=ce[:, sl], func=AF.Exp, scale=-1.0)
        for h in range(2):
            sl = slice(1024 * h, 1024 * (h + 1))
            nc.vector.tensor_scalar(
                out=u_t[:, sl], in0=p_t[:, sl], scalar1=-1.0, scalar2=1.0,
                op0=OP.mult, op1=OP.add)
            nc.vector.tensor_tensor(
                out=v_t[:, sl], in0=u_t[:, sl], in1=u_t[:, sl], op=OP.mult)
            wj = scratch.tile([128, 1024], BF16, tag="wj2", name=f"wj{h}")
            nc.vector.scalar_tensor_tensor(
                out=wj, in0=v_t[:, sl], scalar=0.25, in1=ce[:, sl],
                op0=OP.mult, op1=OP.mult,
                accum_out=acc[:, K_FOCAL + h:K_FOCAL + h + 1])

        # ---------------- partition-reduce + store --------------------
        pfin = psB.tile([1, NPART], F32, tag="sm", name="pfin")
        nc.tensor.matmul(out=pfin, lhsT=ones_f, rhs=acc, start=True, stop=True)
        out_t = tiny.tile([1, NPART], F32, tag="outt")
        nc.vector.tensor_copy(out=out_t, in_=pfin)
        nc.sync.dma_start(out=out[:, :], in_=out_t)


def _zmats():
    """lhsT matrices for row-diff matmuls: out[r] = sum_p Z[p, r] * m[p]."""
    zmA = np.zeros((128, 128), dtype=np.float32)
    zmB = np.zeros((128, 128), dtype=np.float32)
    zmC = np.zeros((128, 128), dtype=np.float32)
    for r in range(127):
        zmA[r + 1, r] = 1.0
        zmA[r, r] = -1.0
        zmC[r + 1, r] = 1.0
        zmC[r, r] = -1.0
    zmA[127, 127] = -1.0   # half0 row127: -m0[127], completed by zmB
    zmB[0, 127] = 1.0      # + m1[0]  -> cross-half boundary diff
    return zmA, zmB, zmC


def _host_inputs(logits, target, features, masks, method_preds):
    """Slice/reshape/convert full inputs into per-core input maps."""
    bf = ml_dtypes.bfloat16
    f8 = ml_dtypes.float8_e4m3
    ident = np.eye(128, dtype=np.float32)
    zmA, zmB, zmC = _zmats()
    cbf = np.ascontiguousarray(
        np.stack([ident, zmA, zmB, zmC], axis=1).astype(bf))  # [128,4,128]
    consts = {"cbf": cbf, "cf32": ident}
    # gather target logit plane on host (pure indexing)
    xt_full = np.take_along_axis(
        logits, target[:, None].astype(np.int64), axis=1)[:, 0]  # [B, H, W]
    in_maps = []
    for c in range(NCORES):
        b0 = c * BP
        lgs = logits[b0:b0 + BP]                                # [4,8,256,256]
        lg_pm = lgs.reshape(BP, C, 128, 512).transpose(2, 0, 1, 3)
        xt_pm = xt_full[b0:b0 + BP].reshape(BP, 128, 512).transpose(1, 0, 2)
        mp_pm = method_preds[:, b0:b0 + BP].reshape(
            3, BP, 128, 512).transpose(2, 0, 1, 3)
        mk_pm = masks[b0:b0 + BP, 0].reshape(BP, 2, 128, 256).transpose(
            2, 0, 1, 3)
        fr = np.roll(features, -c * 128, axis=0)                # [1024, 512]
        ft_pm = fr.T.reshape(4, 128, 1024).transpose(1, 0, 2)
        in_maps.append({
            "lg": np.ascontiguousarray(lg_pm.astype(f8)),
            "xt": np.ascontiguousarray(xt_pm.astype(bf)),
            "mp": np.ascontiguousarray(mp_pm.astype(bf)),
            "mk": np.ascontiguousarray(mk_pm.astype(bf)),
            "ft": np.ascontiguousarray(ft_pm.astype(bf)),
            **consts,
        })
    return in_maps


def _combine(partials):
    """Host-side combination of the per-core [1,32] partial vectors."""
    P = np.stack([np.asarray(p).reshape(-1).astype(np.float64)
                  for p in partials])  # [8,32]
    HW = H * W
    focal = (P[:, K_FOCAL] + P[:, K_FOCAL + 1]).sum() / (B * HW)
    contrast = 0.5 * P[:, K_CONTRAST].sum() / 1024

    circ_total = 0.0
    for c in range(NCORES):
        for b in range(BP):
            area = P[c, K_AREA + b]
            ex = P[c, K_EX + b]
            ey = P[c, K_EY + 2 * b] + P[c, K_EY + 2 * b + 1]
            per = ex + ey
            if area > 0 and per > 0:
                circv = 4.0 * np.pi * area / max(per, 1e-12) ** 2
                circ_total += (circv - 1.0) ** 2
    circ = 0.1 * circ_total / B

    S = P[:, K_S:K_S + 3].sum(axis=0)
    I = P[:, K_I:K_I + 3].sum(axis=0)
    cons_total = 0.0
    for k, (i, j) in enumerate(((0, 1), (0, 2), (1, 2))):
        union = S[i] + S[j] - I[k]
        iou = I[k] / (union + 1e-6)
        cons_total += max(0.6 - iou, 0.0)
    consensus = 0.3 * cons_total / 3.0

    return np.float32(focal + contrast + circ + consensus)


_CACHED_NC = None


def _get_nc():
    global _CACHED_NC
    if _CACHED_NC is None:
        _CACHED_NC = _build_nc()
    return _CACHED_NC


def kernel(logits, target, features, masks, method_preds):
    logits = np.asarray(logits, dtype=np.float32)
    target = np.asarray(target, dtype=np.int32)
    features = np.asarray(features, dtype=np.float32)
    masks = np.asarray(masks, dtype=np.float32)
    method_preds = np.asarray(method_preds, dtype=np.float32)

    in_maps = _host_inputs(logits, target, features, masks, method_preds)
    res = run_bass_kernel_spmd(_get_nc(), in_maps, list(range(NCORES)))
    partials = [res.results[c]["partials"] for c in range(NCORES)]
    return _combine(partials)


# revision 21
# speedup vs baseline: 2.0488x; 1.0593x over previous
"""Trainium2 Bass kernel for CombinedAdvancedLoss (focal + contrastive +
circularity + consensus), data-parallel over 8 NeuronCores.

v3 design:
- logits shipped fp8-e4m3 (2MB/core), everything else bf16, all
  partition-major contiguous. Host gathers the target-logit plane xt
  (pure indexing), so focal is ce = ln(sum_c exp(l_c)) - xt.
- Focal pipelined per image: exp chunk -> 8 accumulating identity matmuls
  into PSUM -> ln -> elementwise tail (2 half-chunks).
- Features shipped pre-transposed+rolled; similarity = unnormalized Gram
  (PE) scaled post-hoc by on-chip row/col inverse norms.
- Work spread across engines: scalar (exp/ln), vector (fused
  scalar_tensor_tensor product-sums, reduces), PE (channel sums, row-diff
  matmuls, column sums), gpsimd (feature squares, row diffs).
- Two DMA queues (SP + Activation) so small inputs land while logits
  stream.

Each core emits a [1,32] vector of linear partial sums; the host combines
them (IoU ratios and the circularity formula act on a handful of scalars).
"""

import sys

for _p in ("/opt/trn_rl_repo",):
    if _p not in sys.path:
        sys.path.insert(0, _p)

import numpy as np
import ml_dtypes

import concourse.bass as bass
import concourse.tile as tile
from concourse import mybir
from concourse.bass_utils import run_bass_kernel_spmd

import bass_rust as _bass_rust

# ---------------------------------------------------------------------------
# The walrus build in this container rejects >2 sync waits per instruction.
# Post-pass: hoist excess waits onto inserted same-engine NoOps.
_WAIT_CAP = 1


def _split_sync_waits(nc):
    n = 0
    for fn in nc.m.functions:
        for blk in fn.blocks:
            insts = blk.instructions
            i = 0
            while i < len(insts):
                inst = insts[i]
                si = inst.sync_info
                if si is not None and len(si.on_wait) > _WAIT_CAP:
                    waits = list(si.on_wait)
                    keep = waits[-_WAIT_CAP:]
                    extra = waits[:-_WAIT_CAP]
                    nops = []
                    for j in range(0, len(extra), _WAIT_CAP):
                        nop = mybir.InstNoOp(
                            name=f"I-wsplit-{n}", engine=inst.engine)
                        n += 1
                        nop.sync_info = _bass_rust.SyncInfo(
                            on_wait=extra[j:j + _WAIT_CAP], on_update=[])
                        nops.append(nop)
                    inst.sync_info = _bass_rust.SyncInfo(
                        on_wait=keep, on_update=list(si.on_update))
                    for k, nop in enumerate(nops):
                        insts.insert(i + k, nop)
                    i += len(nops)
                i += 1
# ---------------------------------------------------------------------------

F32 = mybir.dt.float32
BF16 = mybir.dt.bfloat16
FP8 = mybir.dt.float8e4
AF = mybir.ActivationFunctionType
OP = mybir.AluOpType
AX = mybir.AxisListType

NCORES = 8
B, C, H, W = 32, 8, 256, 256
BP = B // NCORES          # batch items per core (4)
FD = 2048                 # free dim of a full-core pixel tile (BP*512)
TEMP = 0.07
NPART = 32                # width of the per-core partials vector

# partials vector layout
K_FOCAL = 0               # 2 cols (half-chunks): sum 0.25*(1-p)^2 * ce
K_CONTRAST = 2            # sum (lse - pos) over this core's 128 rows
K_AREA = 3                # 4 cols: per-b mask area
K_EX = 7                  # 4 cols: per-b sum |dm/dh| (incl. half boundary)
K_EY = 11                 # 8 cols: per-(b, half) sum |dm/dw|
K_S = 19                  # 3 cols: per-method sum of preds
K_I = 22                  # 3 cols: per-pair sum pi*pj  (01, 02, 12)


def _build_nc():
    nc = bass.Bass()

    lg = nc.declare_dram_parameter("lg", [128, BP, C, 512], FP8, isOutput=False)
    xt = nc.declare_dram_parameter("xt", [128, BP, 512], BF16, isOutput=False)
    mp = nc.declare_dram_parameter("mp", [128, 3, BP, 512], FP8, isOutput=False)
    mk = nc.declare_dram_parameter("mk", [128, BP, 2, 256], FP8, isOutput=False)
    ft = nc.declare_dram_parameter("ft", [128, 4, 1024], BF16, isOutput=False)
    cbf = nc.declare_dram_parameter("cbf", [128, 4, 128], BF16, isOutput=False)
    cf32 = nc.declare_dram_parameter("cf32", [128, 128], F32, isOutput=False)
    out = nc.declare_dram_parameter("partials", [1, NPART], F32, isOutput=True)

    with tile.TileContext(nc) as tc:
        _emit(nc, tc, lg, xt, mp, mk, ft, cbf, cf32, out)
    _split_sync_waits(nc)
    return nc


def _emit(nc, tc, lg, xt, mp, mk, ft, cbf, cf32, out):
    from contextlib import ExitStack

    ctx = ExitStack()
    with ctx:
        singles = ctx.enter_context(tc.tile_pool(name="singles", bufs=1))
        scratch = ctx.enter_context(tc.tile_pool(name="scratch", bufs=2))
        tiny = ctx.enter_context(tc.tile_pool(name="tiny", bufs=1))
        psA = ctx.enter_context(tc.tile_pool(name="psA", bufs=1, space="PSUM"))
        psB = ctx.enter_context(tc.tile_pool(name="psB", bufs=2, space="PSUM"))
        psC = ctx.enter_context(tc.tile_pool(name="psC", bufs=1, space="PSUM"))

        # ---------------- vector-engine constants ----------------
        ones_b = singles.tile([128, 1], BF16)
        nc.vector.memset(ones_b, 1.0)
        ones_f = singles.tile([128, 1], F32)
        nc.vector.memset(ones_f, 1.0)
        acc = singles.tile([128, NPART], F32)
        nc.vector.memset(acc, 0.0)

        # scalar: warm the exp/ln activation-table before any data lands
        warm = tiny.tile([1, 1], F32, tag="warm")
        nc.scalar.activation(out=warm, in_=ones_f[0:1, :], func=AF.Exp)

        # ---------------- DMAs ----------------
        # SP queue: logits chunks (the critical scalar-exp feed), then rest
        lg_t = singles.tile([128, BP, C, 512], FP8)
        nc.sync.dma_start(out=lg_t[:, 0, 0:4], in_=lg[:, 0, 0:4])
        nc.sync.dma_start(out=lg_t[:, 0, 4:8], in_=lg[:, 0, 4:8])
        for j in range(1, BP):
            nc.sync.dma_start(out=lg_t[:, j], in_=lg[:, j])
        mk_t = singles.tile([128, BP, 2, 256], FP8)
        nc.sync.dma_start(out=mk_t, in_=mk[:, :, :, :])
        mp_t = singles.tile([128, 3, BP, 512], FP8)
        nc.sync.dma_start(out=mp_t, in_=mp[:, :, :, :])
        ident_f = singles.tile([128, 128], F32)
        nc.sync.dma_start(out=ident_f, in_=cf32[:, :])
        xt_t = singles.tile([128, BP, 512], BF16)
        nc.sync.dma_start(out=xt_t, in_=xt[:, :, :])
        # Activation queue: consts + features
        cb_t = singles.tile([128, 4, 128], BF16)
        nc.scalar.dma_start(out=cb_t, in_=cbf[:, :, :])
        ft_t = singles.tile([128, 4, 1024], BF16)
        nc.scalar.dma_start(out=ft_t, in_=ft[:, :, :])

        ident_b = cb_t[:, 0]

        # ---------------- feature squares (vector) + row diffs (gp) ---
        sq = singles.tile([128, 4, 1024], BF16)
        nc.vector.tensor_tensor(
            out=sq.rearrange("p a b -> p (a b)"),
            in0=ft_t.rearrange("p a b -> p (a b)"),
            in1=ft_t.rearrange("p a b -> p (a b)"), op=OP.mult)
        d_y = singles.tile([128, BP, 2, 255], BF16)
        nc.gpsimd.tensor_tensor(
            out=d_y, in0=mk_t[:, :, :, 1:256], in1=mk_t[:, :, :, 0:255],
            op=OP.subtract)

        # ---------------- scalar: exps + norms + focal logs ----------
        q = singles.tile([128, BP, C, 512], BF16)
        s_ps = psA.tile([128, 2048], F32, tag="s")

        def exp_chunk(j):
            nc.scalar.activation(
                out=q[:, j].rearrange("p c x -> p (c x)"),
                in_=lg_t[:, j].rearrange("p c x -> p (c x)"),
                func=AF.Exp)

        def smm_chunk(j):
            for c in range(C):
                nc.tensor.matmul(
                    out=s_ps[:, 512 * j:512 * (j + 1)],
                    lhsT=ident_b,
                    rhs=q[:, j, c],
                    start=(c == 0), stop=(c == C - 1))

        nc.scalar.activation(
            out=q[:, 0, 0:4].rearrange("p c x -> p (c x)"),
            in_=lg_t[:, 0, 0:4].rearrange("p c x -> p (c x)"), func=AF.Exp)
        nc.scalar.activation(
            out=q[:, 0, 4:8].rearrange("p c x -> p (c x)"),
            in_=lg_t[:, 0, 4:8].rearrange("p c x -> p (c x)"), func=AF.Exp)
        exp_chunk(1)

        # contrastive part 1: column norms  ss[j] = sum_d ftT[d, j]^2
        ss_ps = psC.tile([1, 1024], F32, tag="g", name="ss_ps")
        for half in range(2):
            for dc in range(4):
                nc.tensor.matmul(
                    out=ss_ps[:, 512 * half:512 * (half + 1)],
                    lhsT=ones_b,
                    rhs=sq[:, dc, 512 * half:512 * (half + 1)],
                    start=(dc == 0), stop=(dc == 3))
        lnss = tiny.tile([1, 1024], F32, tag="lnss")
        nc.scalar.activation(out=lnss, in_=ss_ps, func=AF.Ln)
        colinv = tiny.tile([1, 1024], F32, tag="colinv")
        nc.scalar.activation(out=colinv, in_=lnss, func=AF.Exp, scale=-0.5)

        exp_chunk(2)
        exp_chunk(3)

        # ---------------- PE: focal channel sums + contrastive --------
        smm_chunk(0)
        smm_chunk(1)

        # broadcast colinv down partitions via ones[1,128].T @ colinv
        ones_r = singles.tile([1, 128], F32)
        nc.vector.memset(ones_r, 1.0)
        cb_ps = psC.tile([128, 1024], F32, tag="g", name="cb_ps")
        for half in range(2):
            nc.tensor.matmul(
                out=cb_ps[:, 512 * half:512 * (half + 1)],
                lhsT=ones_r,
                rhs=colinv[:, 512 * half:512 * (half + 1)],
                start=True, stop=True)
        colbc = singles.tile([128, 1024], F32)
        nc.vector.tensor_copy(out=colbc, in_=cb_ps)
        # rowinv = colinv[0:128] transposed, scaled by 1/T
        rT_ps = psB.tile([128, 1], F32, tag="sm", name="rT_ps")
        nc.tensor.transpose(
            out=rT_ps, in_=colinv[:, 0:128], identity=ident_f[0:1, 0:1])
        rowinv = tiny.tile([128, 1], F32, tag="rowinv")
        nc.vector.tensor_scalar(
            out=rowinv, in0=rT_ps, scalar1=1.0 / TEMP, scalar2=None,
            op0=OP.mult)

        # Gram matrix for this core's 128 rows x all 1024 columns
        g_ps = psC.tile([128, 1024], F32, tag="g", name="g_ps")
        for half in range(2):
            for dc in range(4):
                nc.tensor.matmul(
                    out=g_ps[:, 512 * half:512 * (half + 1)],
                    lhsT=ft_t[:, dc, 0:128],
                    rhs=ft_t[:, dc, 512 * half:512 * (half + 1)],
                    start=(dc == 0), stop=(dc == 3))

        smm_chunk(2)

        # circularity row-diff matmuls (3 per image, incl. half boundary)
        for b in range(BP):
            cps = psB.tile([128, 512], F32, tag="sm", name=f"cps{b}")
            nc.tensor.matmul(
                out=cps[:, 0:256], lhsT=cb_t[:, 1], rhs=mk_t[:, b, 0],
                start=True, stop=False)
            nc.tensor.matmul(
                out=cps[:, 0:256], lhsT=cb_t[:, 2], rhs=mk_t[:, b, 1],
                start=False, stop=True)
            nc.tensor.matmul(
                out=cps[:, 256:512], lhsT=cb_t[:, 3], rhs=mk_t[:, b, 1],
                start=True, stop=True)
            nc.vector.tensor_reduce(
                out=acc[:, K_EX + b:K_EX + b + 1], in_=cps,
                axis=AX.XY, op=OP.add, apply_absolute_value=True)

        smm_chunk(3)

        # consensus S_i column sums: ones.T @ mp chunks -> [1, 512]
        for i in range(3):
            sps = psB.tile([1, 512], F32, tag="sm", name=f"sps{i}")
            mflat = mp_t[:, i].rearrange("p b x -> p (b x)")
            for k in range(4):
                nc.tensor.matmul(
                    out=sps,
                    lhsT=ones_b,
                    rhs=mflat[:, 512 * k:512 * (k + 1)],
                    start=(k == 0), stop=(k == 3))
            sjunk = scratch.tile([1, 512], F32, tag="sjk", name=f"sjk{i}")
            nc.vector.tensor_scalar(
                out=sjunk, in0=sps, scalar1=1.0, scalar2=0.0,
                op0=OP.mult, op1=OP.add,
                accum_out=acc[0:1, K_S + i:K_S + i + 1])

        # ---------------- vector: circ/consensus/contrastive ----------
        for b in range(2, BP):
            aj = scratch.tile([128, 512], BF16, tag="aj", name=f"aj{b}")
            nc.vector.tensor_scalar(
                out=aj, in0=mk_t[:, b].rearrange("p h w -> p (h w)"),
                scalar1=1.0, scalar2=0.0, op0=OP.mult, op1=OP.add,
                accum_out=acc[:, K_AREA + b:K_AREA + b + 1])

        st2 = singles.tile([128, 1024], F32)
        nc.vector.tensor_tensor(out=st2, in0=g_ps, in1=colbc, op=OP.mult)
        nc.vector.scalar_tensor_tensor(
            out=st2[:, 0:128], in0=ident_f, scalar=-1e5,
            in1=st2[:, 0:128], op0=OP.mult, op1=OP.add)
        posj = scratch.tile([128, 128], F32, tag="posj")
        posr = tiny.tile([128, 1], F32, tag="posr")
        nc.vector.scalar_tensor_tensor(
            out=posj, in0=st2[:, 512:640], scalar=1.0, in1=ident_f,
            op0=OP.mult, op1=OP.mult, accum_out=posr)

        esim = scratch.tile([128, 1024], BF16, tag="esim")
        rsum = tiny.tile([128, 1], F32, tag="rsum")
        nc.scalar.activation(
            out=esim, in_=st2, func=AF.Exp, scale=rowinv, accum_out=rsum)
        lse = tiny.tile([128, 1], F32, tag="lse")
        nc.scalar.activation(out=lse, in_=rsum, func=AF.Ln)

        # consensus pair intersections (fused product+sum)
        for k, (i, j) in enumerate(((0, 1), (0, 2), (1, 2))):
            ij = scratch.tile([128, 2048], BF16, tag="wj", name=f"ij{k}")
            nc.vector.scalar_tensor_tensor(
                out=ij, in0=mp_t[:, i].rearrange("p b x -> p (b x)"),
                scalar=1.0, in1=mp_t[:, j].rearrange("p b x -> p (b x)"),
                op0=OP.mult, op1=OP.mult,
                accum_out=acc[:, K_I + k:K_I + k + 1])

        # ey: in-row diffs reduce
        nc.vector.tensor_reduce(
            out=acc[:, K_EY:K_EY + 8].rearrange("p (b c) -> p b c", b=BP),
            in_=d_y, axis=AX.X, op=OP.add, apply_absolute_value=True)

        # contrast partial: lse - pos*rowinv
        post = tiny.tile([128, 1], F32, tag="post")
        nc.vector.tensor_scalar(
            out=post, in0=posr, scalar1=rowinv, scalar2=None, op0=OP.mult)
        nc.vector.tensor_tensor(
            out=acc[:, K_CONTRAST:K_CONTRAST + 1], in0=lse, in1=post,
            op=OP.subtract)

        # ---------------- scalar: areas + focal logs ------------------
        for b in range(2):
            aj = scratch.tile([128, 512], BF16, tag="aj", name=f"ajs{b}")
            nc.scalar.activation(
                out=aj, in_=mk_t[:, b].rearrange("p h w -> p (h w)"),
                func=AF.Copy,
                accum_out=acc[:, K_AREA + b:K_AREA + b + 1])

        # ---------------- focal tail (2 half-chunks) ------------------
        ln_s = singles.tile([128, 2048], BF16)
        p_t = singles.tile([128, 2048], BF16)
        ce = singles.tile([128, 2048], BF16)
        u_t = singles.tile([128, 2048], BF16)
        v_t = singles.tile([128, 2048], BF16)
        xtf = xt_t.rearrange("p b x -> p (b x)")
        for h in range(2):
            sl = slice(1024 * h, 1024 * (h + 1))
            nc.scalar.activation(out=ln_s[:, sl], in_=s_ps[:, sl], func=AF.Ln)
        for h in range(2):
            sl = slice(1024 * h, 1024 * (h + 1))
            nc.vector.tensor_tensor(
                out=ce[:, sl], in0=ln_s[:, sl], in1=xtf[:, sl],
                op=OP.subtract)
        for h in range(2):
            sl = slice(1024 * h, 1024 * (h + 1))
            nc.scalar.activation(
                out=p_t[:, sl], in_=ce[:, sl], func=AF.Exp, scale=-1.0)
        for h in range(2):
            sl = slice(1024 * h, 1024 * (h + 1))
            nc.vector.tensor_scalar(
                out=u_t[:, sl], in0=p_t[:, sl], scalar1=-1.0, scalar2=1.0,
                op0=OP.mult, op1=OP.add)
            nc.vector.tensor_tensor(
                out=v_t[:, sl], in0=u_t[:, sl], in1=u_t[:, sl], op=OP.mult)
            wj = scratch.tile([128, 1024], BF16, tag="wj2", name=f"wj{h}")
            nc.vector.scalar_tensor_tensor(
                out=wj, in0=v_t[:, sl], scalar=0.25, in1=ce[:, sl],
                op0=OP.mult, op1=OP.mult,
                accum_out=acc[:, K_FOCAL + h:K_FOCAL + h + 1])

        # ---------------- partition-reduce + store --------------------
        pfin = psB.tile([1, NPART], F32, tag="sm", name="pfin")
        nc.tensor.matmul(out=pfin, lhsT=ones_f, rhs=acc, start=True, stop=True)
        out_t = tiny.tile([1, NPART], F32, tag="outt")
        nc.vector.tensor_copy(out=out_t, in_=pfin)
        nc.sync.dma_start(out=out[:, :], in_=out_t)


def _zmats():
    """lhsT matrices for row-diff matmuls: out[r] = sum_p Z[p, r] * m[p]."""
    zmA = np.zeros((128, 128), dtype=np.float32)
    zmB = np.zeros((128, 128), dtype=np.float32)
    zmC = np.zeros((128, 128), dtype=np.float32)
    for r in range(127):
        zmA[r + 1, r] = 1.0
        zmA[r, r] = -1.0
        zmC[r + 1, r] = 1.0
        zmC[r, r] = -1.0
    zmA[127, 127] = -1.0   # half0 row127: -m0[127], completed by zmB
    zmB[0, 127] = 1.0      # + m1[0]  -> cross-half boundary diff
    return zmA, zmB, zmC


def _host_inputs(logits, target, features, masks, method_preds):
    """Slice/reshape/convert full inputs into per-core input maps."""
    bf = ml_dtypes.bfloat16
    f8 = ml_dtypes.float8_e4m3
    ident = np.eye(128, dtype=np.float32)
    zmA, zmB, zmC = _zmats()
    cbf = np.ascontiguousarray(
        np.stack([ident, zmA, zmB, zmC], axis=1).astype(bf))  # [128,4,128]
    consts = {"cbf": cbf, "cf32": ident}
    # gather target logit plane on host (pure indexing)
    xt_full = np.take_along_axis(
        logits, target[:, None].astype(np.int64), axis=1)[:, 0]  # [B, H, W]
    in_maps = []
    for c in range(NCORES):
        b0 = c * BP
        lgs = logits[b0:b0 + BP]                                # [4,8,256,256]
        lg_pm = lgs.reshape(BP, C, 128, 512).transpose(2, 0, 1, 3)
        xt_pm = xt_full[b0:b0 + BP].reshape(BP, 128, 512).transpose(1, 0, 2)
        mp_pm = method_preds[:, b0:b0 + BP].reshape(
            3, BP, 128, 512).transpose(2, 0, 1, 3)
        mk_pm = masks[b0:b0 + BP, 0].reshape(BP, 2, 128, 256).transpose(
            2, 0, 1, 3)
        fr = np.roll(features, -c * 128, axis=0)                # [1024, 512]
        ft_pm = fr.T.reshape(4, 128, 1024).transpose(1, 0, 2)
        in_maps.append({
            "lg": np.ascontiguousarray(lg_pm.astype(f8)),
            "xt": np.ascontiguousarray(xt_pm.astype(bf)),
            "mp": np.ascontiguousarray(mp_pm.astype(f8)),
            "mk": np.ascontiguousarray(mk_pm.astype(f8)),
            "ft": np.ascontiguousarray(ft_pm.astype(bf)),
            **consts,
        })
    return in_maps


def _combine(partials):
    """Host-side combination of the per-core [1,32] partial vectors."""
    P = np.stack([np.asarray(p).reshape(-1).astype(np.float64)
                  for p in partials])  # [8,32]
    HW = H * W
    focal = (P[:, K_FOCAL] + P[:, K_FOCAL + 1]).sum() / (B * HW)
    contrast = 0.5 * P[:, K_CONTRAST].sum() / 1024

    circ_total = 0.0
    for c in range(NCORES):
        for b in range(BP):
            area = P[c, K_AREA + b]
            ex = P[c, K_EX + b]
            ey = P[c, K_EY + 2 * b] + P[c, K_EY + 2 * b + 1]
            per = ex + ey
            if area > 0 and per > 0:
                circv = 4.0 * np.pi * area / max(per, 1e-12) ** 2
                circ_total += (circv - 1.0) ** 2
    circ = 0.1 * circ_total / B

    S = P[:, K_S:K_S + 3].sum(axis=0)
    I = P[:, K_I:K_I + 3].sum(axis=0)
    cons_total = 0.0
    for k, (i, j) in enumerate(((0, 1), (0, 2), (1, 2))):
        union = S[i] + S[j] - I[k]
        iou = I[k] / (union + 1e-6)
        cons_total += max(0.6 - iou, 0.0)
    consensus = 0.3 * cons_total / 3.0

    return np.float32(focal + contrast + circ + consensus)


_CACHED_NC = None


def _get_nc():
    global _CACHED_NC
    if _CACHED_NC is None:
        _CACHED_NC = _build_nc()
    return _CACHED_NC


def kernel(logits, target, features, masks, method_preds):
    logits = np.asarray(logits, dtype=np.float32)
    target = np.asarray(target, dtype=np.int32)
    features = np.asarray(features, dtype=np.float32)
    masks = np.asarray(masks, dtype=np.float32)
    method_preds = np.asarray(method_preds, dtype=np.float32)

    in_maps = _host_inputs(logits, target, features, masks, method_preds)
    res = run_bass_kernel_spmd(_get_nc(), in_maps, list(range(NCORES)))
    partials = [res.results[c]["partials"] for c in range(NCORES)]
    return _combine(partials)


# revision 27
# speedup vs baseline: 2.2703x; 1.1081x over previous
"""Trainium2 Bass kernel for CombinedAdvancedLoss (focal + contrastive +
circularity + consensus), data-parallel over 8 NeuronCores.

v3 design:
- logits shipped fp8-e4m3 (2MB/core), everything else bf16, all
  partition-major contiguous. Host gathers the target-logit plane xt
  (pure indexing), so focal is ce = ln(sum_c exp(l_c)) - xt.
- Focal pipelined per image: exp chunk -> 8 accumulating identity matmuls
  into PSUM -> ln -> elementwise tail (2 half-chunks).
- Features shipped pre-transposed+rolled; similarity = unnormalized Gram
  (PE) scaled post-hoc by on-chip row/col inverse norms.
- Work spread across engines: scalar (exp/ln), vector (fused
  scalar_tensor_tensor product-sums, reduces), PE (channel sums, row-diff
  matmuls, column sums), gpsimd (feature squares, row diffs).
- Two DMA queues (SP + Activation) so small inputs land while logits
  stream.

Each core emits a [1,32] vector of linear partial sums; the host combines
them (IoU ratios and the circularity formula act on a handful of scalars).
"""

import sys

for _p in ("/opt/trn_rl_repo",):
    if _p not in sys.path:
        sys.path.insert(0, _p)

import numpy as np
import ml_dtypes

import concourse.bass as bass
import concourse.tile as tile
from concourse import mybir
from concourse.bass_utils import run_bass_kernel_spmd

import bass_rust as _bass_rust

# ---------------------------------------------------------------------------
# The walrus build in this container rejects >2 sync waits per instruction.
# Post-pass: hoist excess waits onto inserted same-engine NoOps.
_WAIT_CAP = 1


def _split_sync_waits(nc):
    n = 0
    for fn in nc.m.functions:
        for blk in fn.blocks:
            insts = blk.instructions
            i = 0
            while i < len(insts):
                inst = insts[i]
                si = inst.sync_info
                if si is not None and len(si.on_wait) > _WAIT_CAP:
                    waits = list(si.on_wait)
                    keep = waits[-_WAIT_CAP:]
                    extra = waits[:-_WAIT_CAP]
                    nops = []
                    for j in range(0, len(extra), _WAIT_CAP):
                        nop = mybir.InstNoOp(
                            name=f"I-wsplit-{n}", engine=inst.engine)
                        n += 1
                        nop.sync_info = _bass_rust.SyncInfo(
                            on_wait=extra[j:j + _WAIT_CAP], on_update=[])
                        nops.append(nop)
                    inst.sync_info = _bass_rust.SyncInfo(
                        on_wait=keep, on_update=list(si.on_update))
                    for k, nop in enumerate(nops):
                        insts.insert(i + k, nop)
                    i += len(nops)
                i += 1
# ---------------------------------------------------------------------------

F32 = mybir.dt.float32
BF16 = mybir.dt.bfloat16
FP8 = mybir.dt.float8e3
AF = mybir.ActivationFunctionType
OP = mybir.AluOpType
AX = mybir.AxisListType

NCORES = 8
B, C, H, W = 32, 8, 256, 256
BP = B // NCORES          # batch items per core (4)
FD = 2048                 # free dim of a full-core pixel tile (BP*512)
TEMP = 0.07
NPART = 32                # width of the per-core partials vector

# partials vector layout
K_FOCAL = 0               # 2 cols (half-chunks): sum 0.25*(1-p)^2 * ce
K_CONTRAST = 2            # sum (lse - pos) over this core's 128 rows
K_AREA = 3                # 4 cols: per-b mask area
K_EX = 7                  # 4 cols: per-b sum |dm/dh| (incl. half boundary)
K_EY = 11                 # 8 cols: per-(b, half) sum |dm/dw|
K_S = 19                  # 3 cols: per-method sum of preds
K_I = 22                  # 3 cols: per-pair sum pi*pj  (01, 02, 12)


def _build_nc():
    nc = bass.Bass()

    lg = nc.declare_dram_parameter("lg", [128, BP, C, 512], FP8, isOutput=False)
    xt = nc.declare_dram_parameter("xt", [128, BP, 512], FP8, isOutput=False)
    mp = nc.declare_dram_parameter("mp", [128, 3, BP, 512], FP8, isOutput=False)
    mk = nc.declare_dram_parameter("mk", [128, BP, 2, 256], FP8, isOutput=False)
    ft = nc.declare_dram_parameter("ft", [128, 4, 1024], FP8, isOutput=False)
    cbf = nc.declare_dram_parameter("cbf", [128, 4, 128], BF16, isOutput=False)
    cf32 = nc.declare_dram_parameter("cf32", [128, 128], F32, isOutput=False)
    out = nc.declare_dram_parameter("partials", [1, NPART], F32, isOutput=True)

    with tile.TileContext(nc) as tc:
        _emit(nc, tc, lg, xt, mp, mk, ft, cbf, cf32, out)
    _split_sync_waits(nc)
    return nc


def _emit(nc, tc, lg, xt, mp, mk, ft, cbf, cf32, out):
    from contextlib import ExitStack

    ctx = ExitStack()
    with ctx:
        singles = ctx.enter_context(tc.tile_pool(name="singles", bufs=1))
        scratch = ctx.enter_context(tc.tile_pool(name="scratch", bufs=2))
        tiny = ctx.enter_context(tc.tile_pool(name="tiny", bufs=1))
        psA = ctx.enter_context(tc.tile_pool(name="psA", bufs=1, space="PSUM"))
        psB = ctx.enter_context(tc.tile_pool(name="psB", bufs=2, space="PSUM"))
        psC = ctx.enter_context(tc.tile_pool(name="psC", bufs=1, space="PSUM"))

        # ---------------- vector-engine constants ----------------
        ones_b = singles.tile([128, 1], BF16)
        nc.vector.memset(ones_b, 1.0)
        ones_f = singles.tile([128, 1], F32)
        nc.vector.memset(ones_f, 1.0)
        ones_r = singles.tile([1, 128], F32)
        nc.vector.memset(ones_r, 1.0)
        acc = singles.tile([128, NPART], F32)
        nc.vector.memset(acc, 0.0)

        # scalar: warm the exp/ln activation-table before any data lands
        warm = tiny.tile([1, 1], F32, tag="warm")
        nc.scalar.activation(out=warm, in_=ones_f[0:1, :], func=AF.Exp)

        # ---------------- DMAs ----------------
        # SP queue: logits chunks (the critical scalar-exp feed) + xt
        lg_t = singles.tile([128, BP, C, 512], FP8)
        nc.sync.dma_start(out=lg_t[:, 0, 0:4], in_=lg[:, 0, 0:4])
        nc.sync.dma_start(out=lg_t[:, 0, 4:8], in_=lg[:, 0, 4:8])
        for j in range(1, BP):
            nc.sync.dma_start(out=lg_t[:, j], in_=lg[:, j])
        xt_t = singles.tile([128, BP, 512], FP8)
        nc.sync.dma_start(out=xt_t, in_=xt[:, :, :])
        # Activation queue: consts, features, masks, method preds
        cb_t = singles.tile([128, 4, 128], BF16)
        nc.scalar.dma_start(out=cb_t, in_=cbf[:, :, :])
        ft_t = singles.tile([128, 4, 1024], FP8)
        nc.scalar.dma_start(out=ft_t, in_=ft[:, :, :])
        mk_t = singles.tile([128, BP, 2, 256], FP8)
        nc.scalar.dma_start(out=mk_t, in_=mk[:, :, :, :])
        mp_t = singles.tile([128, 3, BP, 512], FP8)
        nc.scalar.dma_start(out=mp_t, in_=mp[:, :, :, :])
        ident_f = singles.tile([128, 128], F32)
        nc.scalar.dma_start(out=ident_f, in_=cf32[:, :])

        ident_b = cb_t[:, 0]

        # ---------------- feature squares (split) + row diffs ---------
        sq = singles.tile([128, 4, 1024], BF16)
        nc.vector.tensor_tensor(
            out=sq[:, 0:2].rearrange("p a b -> p (a b)"),
            in0=ft_t[:, 0:2].rearrange("p a b -> p (a b)"),
            in1=ft_t[:, 0:2].rearrange("p a b -> p (a b)"), op=OP.mult)
        nc.gpsimd.tensor_tensor(
            out=sq[:, 2:4].rearrange("p a b -> p (a b)"),
            in0=ft_t[:, 2:4].rearrange("p a b -> p (a b)"),
            in1=ft_t[:, 2:4].rearrange("p a b -> p (a b)"), op=OP.mult)
        d_y = singles.tile([128, BP, 2, 255], BF16)
        nc.gpsimd.tensor_tensor(
            out=d_y, in0=mk_t[:, :, :, 1:256], in1=mk_t[:, :, :, 0:255],
            op=OP.subtract)

        # ---------------- scalar: exp spine ---------------------------
        q = singles.tile([128, BP, C, 512], BF16)
        s_ps = psA.tile([128, 2048], F32, tag="s")

        def exp_chunk(j):
            nc.scalar.activation(
                out=q[:, j].rearrange("p c x -> p (c x)"),
                in_=lg_t[:, j].rearrange("p c x -> p (c x)"),
                func=AF.Exp)

        def smm_chunk(j):
            for c in range(C):
                nc.tensor.matmul(
                    out=s_ps[:, 512 * j:512 * (j + 1)],
                    lhsT=ident_b,
                    rhs=q[:, j, c],
                    start=(c == 0), stop=(c == C - 1))

        nc.scalar.activation(
            out=q[:, 0, 0:4].rearrange("p c x -> p (c x)"),
            in_=lg_t[:, 0, 0:4].rearrange("p c x -> p (c x)"), func=AF.Exp)
        nc.scalar.activation(
            out=q[:, 0, 4:8].rearrange("p c x -> p (c x)"),
            in_=lg_t[:, 0, 4:8].rearrange("p c x -> p (c x)"), func=AF.Exp)
        exp_chunk(1)
        exp_chunk(2)

        # contrastive norms: ss column sums on PE, logs squeezed into the
        # exp spine (deps land well before exp2 completes)
        smm_chunk(0)
        smm_chunk(1)
        ss_ps = psC.tile([1, 1024], F32, tag="g", name="ss_ps")
        for half in range(2):
            for dc in range(4):
                nc.tensor.matmul(
                    out=ss_ps[:, 512 * half:512 * (half + 1)],
                    lhsT=ones_b,
                    rhs=sq[:, dc, 512 * half:512 * (half + 1)],
                    start=(dc == 0), stop=(dc == 3))
        lnss = tiny.tile([1, 1024], F32, tag="lnss")
        nc.scalar.activation(out=lnss, in_=ss_ps, func=AF.Ln)
        colinv = tiny.tile([1, 1024], F32, tag="colinv")
        nc.scalar.activation(out=colinv, in_=lnss, func=AF.Exp, scale=-0.5)

        exp_chunk(3)

        # ---------------- PE program --------------------------------
        smm_chunk(2)
        g_ps = psC.tile([128, 1024], F32, tag="g", name="g_ps")
        for half in range(2):
            for dc in range(4):
                nc.tensor.matmul(
                    out=g_ps[:, 512 * half:512 * (half + 1)],
                    lhsT=ft_t[:, dc, 0:128],
                    rhs=ft_t[:, dc, 512 * half:512 * (half + 1)],
                    start=(dc == 0), stop=(dc == 3))
        # circularity row-diff matmuls (3 per image, incl. half boundary)
        for b in range(BP):
            cps = psB.tile([128, 512], F32, tag="sm", name=f"cps{b}")
            nc.tensor.matmul(
                out=cps[:, 0:256], lhsT=cb_t[:, 1], rhs=mk_t[:, b, 0],
                start=True, stop=False)
            nc.tensor.matmul(
                out=cps[:, 0:256], lhsT=cb_t[:, 2], rhs=mk_t[:, b, 1],
                start=False, stop=True)
            nc.tensor.matmul(
                out=cps[:, 256:512], lhsT=cb_t[:, 3], rhs=mk_t[:, b, 1],
                start=True, stop=True)
            nc.vector.tensor_reduce(
                out=acc[:, K_EX + b:K_EX + b + 1], in_=cps,
                axis=AX.XY, op=OP.add, apply_absolute_value=True)
        # colinv broadcast into two psB half-slots (avoids psC ring cycle)
        cbh = []
        for half in range(2):
            cbp = psB.tile([128, 512], F32, tag="sm", name=f"cbp{half}")
            nc.tensor.matmul(
                out=cbp, lhsT=ones_r,
                rhs=colinv[:, 512 * half:512 * (half + 1)],
                start=True, stop=True)
            cbh.append(cbp)
        smm_chunk(3)
        # consensus S_i column sums: ones.T @ mp chunks -> [1, 512]
        for i in range(3):
            sps = psB.tile([1, 512], F32, tag="sm", name=f"sps{i}")
            mflat = mp_t[:, i].rearrange("p b x -> p (b x)")
            for k in range(4):
                nc.tensor.matmul(
                    out=sps,
                    lhsT=ones_b,
                    rhs=mflat[:, 512 * k:512 * (k + 1)],
                    start=(k == 0), stop=(k == 3))
            sjunk = scratch.tile([1, 512], F32, tag="sjk", name=f"sjk{i}")
            nc.vector.tensor_scalar(
                out=sjunk, in0=sps, scalar1=1.0, scalar2=0.0,
                op0=OP.mult, op1=OP.add,
                accum_out=acc[0:1, K_S + i:K_S + i + 1])
        # rowinv = colinv[0:128] transposed, scaled by 1/T
        rT_ps = psB.tile([128, 1], F32, tag="sm", name="rT_ps")
        nc.tensor.transpose(
            out=rT_ps, in_=colinv[:, 0:128], identity=ident_f[0:1, 0:1])

        # ---------------- vector program ------------------------------
        for b in range(BP):
            aj = scratch.tile([128, 512], BF16, tag="aj", name=f"aj{b}")
            nc.vector.tensor_scalar(
                out=aj, in0=mk_t[:, b].rearrange("p h w -> p (h w)"),
                scalar1=1.0, scalar2=0.0, op0=OP.mult, op1=OP.add,
                accum_out=acc[:, K_AREA + b:K_AREA + b + 1])
        # consensus pair intersections (fused product+sum)
        for k, (i, j) in enumerate(((0, 1), (0, 2), (1, 2))):
            ij = scratch.tile([128, 2048], BF16, tag="wj", name=f"ij{k}")
            nc.vector.scalar_tensor_tensor(
                out=ij, in0=mp_t[:, i].rearrange("p b x -> p (b x)"),
                scalar=1.0, in1=mp_t[:, j].rearrange("p b x -> p (b x)"),
                op0=OP.mult, op1=OP.mult,
                accum_out=acc[:, K_I + k:K_I + k + 1])
        # colinv broadcast to SBUF + rowinv
        colbc = singles.tile([128, 1024], F32)
        nc.vector.tensor_copy(out=colbc[:, 0:512], in_=cbh[0])
        nc.vector.tensor_copy(out=colbc[:, 512:1024], in_=cbh[1])
        rowinv = tiny.tile([128, 1], F32, tag="rowinv")
        nc.vector.tensor_scalar(
            out=rowinv, in0=rT_ps, scalar1=1.0 / TEMP, scalar2=None,
            op0=OP.mult)
        st2 = singles.tile([128, 1024], F32)
        nc.vector.tensor_tensor(out=st2, in0=g_ps, in1=colbc, op=OP.mult)
        nc.vector.scalar_tensor_tensor(
            out=st2[:, 0:128], in0=ident_f, scalar=-1e5,
            in1=st2[:, 0:128], op0=OP.mult, op1=OP.add)
        posj = scratch.tile([128, 128], F32, tag="posj")
        posr = tiny.tile([128, 1], F32, tag="posr")
        nc.vector.scalar_tensor_tensor(
            out=posj, in0=st2[:, 512:640], scalar=1.0, in1=ident_f,
            op0=OP.mult, op1=OP.mult, accum_out=posr)
        # ey: in-row diffs reduce
        nc.vector.tensor_reduce(
            out=acc[:, K_EY:K_EY + 8].rearrange("p (b c) -> p b c", b=BP),
            in_=d_y, axis=AX.X, op=OP.add, apply_absolute_value=True)

        # ---------------- scalar: contrastive exp + areas + logs -----
        esim = scratch.tile([128, 1024], BF16, tag="esim")
        rsum = tiny.tile([128, 1], F32, tag="rsum")
        nc.scalar.activation(
            out=esim, in_=st2, func=AF.Exp, scale=rowinv, accum_out=rsum)
        lse = tiny.tile([128, 1], F32, tag="lse")
        nc.scalar.activation(out=lse, in_=rsum, func=AF.Ln)
        # ---------------- focal tail (2 half-chunks) ------------------
        ln_s = singles.tile([128, 2048], BF16)
        p_t = singles.tile([128, 2048], BF16)
        ce = singles.tile([128, 2048], BF16)
        u_t = singles.tile([128, 2048], BF16)
        v_t = singles.tile([128, 2048], BF16)
        xtf = xt_t.rearrange("p b x -> p (b x)")
        for h in range(2):
            sl = slice(1024 * h, 1024 * (h + 1))
            nc.scalar.activation(out=ln_s[:, sl], in_=s_ps[:, sl], func=AF.Ln)
        # contrast partial: lse - pos*rowinv
        post = tiny.tile([128, 1], F32, tag="post")
        nc.vector.tensor_scalar(
            out=post, in0=posr, scalar1=rowinv, scalar2=None, op0=OP.mult)
        nc.vector.tensor_tensor(
            out=acc[:, K_CONTRAST:K_CONTRAST + 1], in0=lse, in1=post,
            op=OP.subtract)
        for h in range(2):
            sl = slice(1024 * h, 1024 * (h + 1))
            nc.vector.tensor_tensor(
                out=ce[:, sl], in0=ln_s[:, sl], in1=xtf[:, sl],
                op=OP.subtract)
        for h in range(2):
            sl = slice(1024 * h, 1024 * (h + 1))
            nc.scalar.activation(
                out=p_t[:, sl], in_=ce[:, sl], func=AF.Exp, scale=-1.0)
        for h in range(2):
            sl = slice(1024 * h, 1024 * (h + 1))
            nc.vector.tensor_scalar(
                out=u_t[:, sl], in0=p_t[:, sl], scalar1=-1.0, scalar2=1.0,
                op0=OP.mult, op1=OP.add)
            nc.vector.tensor_tensor(
                out=v_t[:, sl], in0=u_t[:, sl], in1=u_t[:, sl], op=OP.mult)
            wj = scratch.tile([128, 1024], BF16, tag="wj2", name=f"wj{h}")
            nc.vector.scalar_tensor_tensor(
                out=wj, in0=v_t[:, sl], scalar=0.25, in1=ce[:, sl],
                op0=OP.mult, op1=OP.mult,
                accum_out=acc[:, K_FOCAL + h:K_FOCAL + h + 1])

        # ---------------- partition-reduce + store --------------------
        pfin = psB.tile([1, NPART], F32, tag="sm", name="pfin")
        nc.tensor.matmul(out=pfin, lhsT=ones_f, rhs=acc, start=True, stop=True)
        out_t = tiny.tile([1, NPART], F32, tag="outt")
        nc.vector.tensor_copy(out=out_t, in_=pfin)
        nc.scalar.dma_start(out=out[:, :], in_=out_t)


def _zmats():
    """lhsT matrices for row-diff matmuls: out[r] = sum_p Z[p, r] * m[p]."""
    zmA = np.zeros((128, 128), dtype=np.float32)
    zmB = np.zeros((128, 128), dtype=np.float32)
    zmC = np.zeros((128, 128), dtype=np.float32)
    for r in range(127):
        zmA[r + 1, r] = 1.0
        zmA[r, r] = -1.0
        zmC[r + 1, r] = 1.0
        zmC[r, r] = -1.0
    zmA[127, 127] = -1.0   # half0 row127: -m0[127], completed by zmB
    zmB[0, 127] = 1.0      # + m1[0]  -> cross-half boundary diff
    return zmA, zmB, zmC


def _host_inputs(logits, target, features, masks, method_preds):
    """Slice/reshape/convert full inputs into per-core input maps."""
    bf = ml_dtypes.bfloat16
    f8 = ml_dtypes.float8_e3m4
    ident = np.eye(128, dtype=np.float32)
    zmA, zmB, zmC = _zmats()
    cbf = np.ascontiguousarray(
        np.stack([ident, zmA, zmB, zmC], axis=1).astype(bf))  # [128,4,128]
    consts = {"cbf": cbf, "cf32": ident}
    # gather target logit plane on host (pure indexing)
    xt_full = np.take_along_axis(
        logits, target[:, None].astype(np.int64), axis=1)[:, 0]  # [B, H, W]
    in_maps = []
    for c in range(NCORES):
        b0 = c * BP
        lgs = logits[b0:b0 + BP]                                # [4,8,256,256]
        lg_pm = lgs.reshape(BP, C, 128, 512).transpose(2, 0, 1, 3)
        xt_pm = xt_full[b0:b0 + BP].reshape(BP, 128, 512).transpose(1, 0, 2)
        mp_pm = method_preds[:, b0:b0 + BP].reshape(
            3, BP, 128, 512).transpose(2, 0, 1, 3)
        mk_pm = masks[b0:b0 + BP, 0].reshape(BP, 2, 128, 256).transpose(
            2, 0, 1, 3)
        fr = np.roll(features, -c * 128, axis=0)                # [1024, 512]
        ft_pm = fr.T.reshape(4, 128, 1024).transpose(1, 0, 2)
        in_maps.append({
            "lg": np.ascontiguousarray(lg_pm.astype(f8)),
            "xt": np.ascontiguousarray(xt_pm.astype(f8)),
            "mp": np.ascontiguousarray(mp_pm.astype(f8)),
            "mk": np.ascontiguousarray(mk_pm.astype(f8)),
            "ft": np.ascontiguousarray(ft_pm.astype(f8)),
            **consts,
        })
    return in_maps


def _combine(partials):
    """Host-side combination of the per-core [1,32] partial vectors."""
    P = np.stack([np.asarray(p).reshape(-1).astype(np.float64)
                  for p in partials])  # [8,32]
    HW = H * W
    focal = (P[:, K_FOCAL] + P[:, K_FOCAL + 1]).sum() / (B * HW)
    contrast = 0.5 * P[:, K_CONTRAST].sum() / 1024

    circ_total = 0.0
    for c in range(NCORES):
        for b in range(BP):
            area = P[c, K_AREA + b]
            ex = P[c, K_EX + b]
            ey = P[c, K_EY + 2 * b] + P[c, K_EY + 2 * b + 1]
            per = ex + ey
            if area > 0 and per > 0:
                circv = 4.0 * np.pi * area / max(per, 1e-12) ** 2
                circ_total += (circv - 1.0) ** 2
    circ = 0.1 * circ_total / B

    S = P[:, K_S:K_S + 3].sum(axis=0)
    I = P[:, K_I:K_I + 3].sum(axis=0)
    cons_total = 0.0
    for k, (i, j) in enumerate(((0, 1), (0, 2), (1, 2))):
        union = S[i] + S[j] - I[k]
        iou = I[k] / (union + 1e-6)
        cons_total += max(0.6 - iou, 0.0)
    consensus = 0.3 * cons_total / 3.0

    return np.float32(focal + contrast + circ + consensus)


_CACHED_NC = None


def _get_nc():
    global _CACHED_NC
    if _CACHED_NC is None:
        _CACHED_NC = _build_nc()
    return _CACHED_NC


def kernel(logits, target, features, masks, method_preds):
    logits = np.asarray(logits, dtype=np.float32)
    target = np.asarray(target, dtype=np.int32)
    features = np.asarray(features, dtype=np.float32)
    masks = np.asarray(masks, dtype=np.float32)
    method_preds = np.asarray(method_preds, dtype=np.float32)

    in_maps = _host_inputs(logits, target, features, masks, method_preds)
    res = run_bass_kernel_spmd(_get_nc(), in_maps, list(range(NCORES)))
    partials = [res.results[c]["partials"] for c in range(NCORES)]
    return _combine(partials)
